# revision 13
# baseline (speedup 1.0000x reference)
"""Trainium2 Bass kernel for nn_Decoder (Tacotron-style LSTM encoder/decoder).

Architecture (8 NeuronCores, data-parallel over batch N=64 -> 8/core):
  - Transposed layout: hidden dim on SBUF partitions, (time,batch) on the
    free dim; input-side projections batched into large matmuls per 64-step
    chunk; only h @ Whh.T stays per-step.
  - Recurrent h state lives in ping-pong tiles with STATIC addresses (the
    2-step-unrolled For_i body alternates them), so the 16 per-step h-matmuls
    need no dynamic address computation on the PE sequencer.  The h history
    (consumed at chunk granularity by the next layer / FC) is written by the
    otherwise-idle GpSimd engine, off the recurrent critical path.
  - Gate order is host-permuted to [i,f,o,g] so one sigmoid covers i,f,o and
    one tanh covers g: 3 activation instructions per step instead of 4.
  - Activation tables are pinned to the single set containing sigmoid+tanh+
    copy, eliminating per-iteration act-table reloads.
  - Decoder runs a 4-layer chunk-lagged wavefront; the elementwise tails are
    merged across groups of 2 layers (one instruction covers both layers'
    gates), halving tail instruction count while keeping 2 independent
    dependency chains in flight.
  - Weights / h / x-projections in bf16, cell state c + PSUM in fp32.
"""

import numpy as np
import ml_dtypes

H = 256
NMEL = 80
D_ENC = 512
NCORES = 8
NL = 8          # batch per core
C = 64          # chunk (time) size

# new j-block order [i0,i1,f0,f1,o0,o1,g0,g1] <- old [i,i,f,f,g,g,o,o]
GPERM = [0, 1, 2, 3, 6, 7, 4, 5]

_prog_cache = {}


def _build_program(S, T):
    import concourse.bass as bass
    import concourse.mybir as mybir
    import concourse.tile as tile
    import concourse.bacc as bacc_mod
    from concourse import bacc
    from concourse.bass import ds
    from concourse.masks import make_identity
    from contextlib import ExitStack

    BF = mybir.dt.bfloat16
    FP = mybir.dt.float32
    A = mybir.ActivationFunctionType

    TD = T
    assert S % C == 0 and TD % C == 0
    SC = S // C
    DC = TD // C

    nc = bacc.Bacc("TRN2", target_bir_lowering=False, debug=False,
                   num_devices=NCORES)

    # ---------------- DRAM I/O ----------------
    d_encrhs = nc.dram_tensor("encrhs", [4, 128, S, NL], BF, kind="ExternalInput").ap()
    d_mels = nc.dram_tensor("mels", [NMEL, T, NL], FP, kind="ExternalInput").ap()
    d_ictx = nc.dram_tensor("ictx", [NMEL], FP, kind="ExternalInput").ap()
    d_ewih = nc.dram_tensor("ewih", [128, 128, 128], BF, kind="ExternalInput").ap()
    d_ewhh = nc.dram_tensor("ewhh", [128, 64, 128], BF, kind="ExternalInput").ap()
    d_eb = nc.dram_tensor("eb", [1, 32, 128], BF, kind="ExternalInput").ap()
    d_dwih0 = nc.dram_tensor("dwih0", [97, 8, 128], BF, kind="ExternalInput").ap()
    d_dwih = nc.dram_tensor("dwih", [128, 48, 128], BF, kind="ExternalInput").ap()
    d_db = nc.dram_tensor("db", [1, 24, 128], BF, kind="ExternalInput").ap()
    d_dwhh = nc.dram_tensor("dwhh", [128, 64, 128], BF, kind="ExternalInput").ap()
    d_fcw = nc.dram_tensor("fcw", [128, 2, NMEL], BF, kind="ExternalInput").ap()
    d_fcb = nc.dram_tensor("fcb", [1, NMEL], BF, kind="ExternalInput").ap()
    d_out = nc.dram_tensor("out", [NMEL, T, NL], FP, kind="ExternalOutput").ap()

    CB = C * NL  # tokens per chunk = 512

    with tile.TileContext(nc) as tc:
        with ExitStack() as ctx:
            persist = ctx.enter_context(tc.tile_pool(name="persist", bufs=1))
            psum_x = ctx.enter_context(
                tc.tile_pool(name="psx", bufs=2, space="PSUM"))
            stash = ctx.enter_context(tc.tile_pool(name="stash", bufs=3))

            ident = persist.tile([128, 128], BF)
            make_identity(nc, ident)
            ones = persist.tile([1, CB], BF)
            nc.vector.memset(ones, 1.0)

            hinit = [persist.tile([128, 2, NL], BF, name=f"hinit{l}")
                     for l in range(4)]
            cinit = [persist.tile([128, 2, NL], FP, name=f"cinit{l}")
                     for l in range(4)]

            # ---------- batched input projection for one chunk ----------
            def emit_xtilde(xsrcs, brow, dst, copy_eng):
                """xsrcs: list of (lhsT_ap, rhs_fn) accumulated; brow: bias
                tile ap or None; dst[j]: fn j -> dst ap [128, C*NL]."""
                nk = len(xsrcs)
                for j in range(8):
                    xp = psum_x.tile([128, CB], FP, tag="xp")
                    for kk, (wap, rhs) in enumerate(xsrcs):
                        nc.tensor.matmul(xp, wap(j, kk), rhs,
                                         start=(kk == 0),
                                         stop=False if brow else (kk == nk - 1))
                    if brow:
                        nc.tensor.matmul(xp, brow(j), ones, start=False,
                                         stop=True)
                    eng = copy_eng[j % len(copy_eng)]
                    if eng == "s":
                        nc.scalar.copy(dst(j), xp)
                    elif eng == "v":
                        nc.vector.tensor_copy(dst(j), xp)
                    else:
                        nc.gpsimd.tensor_copy(dst(j), xp)

            # =======================================================
            # ENCODER
            # =======================================================
            with ExitStack() as ectx:
                epool = ectx.enter_context(tc.tile_pool(name="enc", bufs=1))
                psg_e = ectx.enter_context(
                    tc.tile_pool(name="psge", bufs=2, space="PSUM"))
                ew_ih = epool.tile([128, 128, 128], BF)
                ew_hh = epool.tile([128, 64, 128], BF)
                ew_b = epool.tile([1, 32, 128], BF)
                nc.sync.dma_start(out=ew_ih, in_=d_ewih)
                nc.sync.dma_start(out=ew_hh, in_=d_ewhh)
                nc.sync.dma_start(out=ew_b, in_=d_eb)

                eo_bf = epool.tile([128, 4, S, NL], BF)
                for kk in range(4):
                    nc.sync.dma_start(out=eo_bf[:, kk], in_=d_encrhs[kk])

                # history tiles only for layer 0 (consumed by layer 1)
                e_hist = [epool.tile([128, S + 1, 2, NL], BF, name=f"eh{d}")
                          for d in range(2)]

                escan = {}
                for (l, d) in [(0, 0), (0, 1), (1, 0), (1, 1)]:
                    tag = f"e{l}{d}"
                    si = l * 2 + d
                    cst = epool.tile([128, 2, NL], FP, name=f"c_{tag}")
                    hp = [epool.tile([128, 2, NL], BF, name=f"hp{u}_{tag}")
                          for u in range(2)]
                    xsb = epool.tile([128, 8, C, NL], BF, name=f"x_{tag}")
                    nc.vector.memset(cst, 0.0)
                    nc.vector.memset(hp[0], 0.0)
                    escan[tag] = dict(
                        tag=tag, fwd=(d == 0), cst=cst, hp=hp, xsb=xsb,
                        whh_base=si * 16, wih_base=si * 32, b_base=si * 8,
                        hist=e_hist[d] if l == 0 else None)

                def enc_xtilde(sc, l, d, k):
                    tr0 = k * C if d == 0 else S - (k + 1) * C
                    srcs = []
                    for kk in range(4):
                        if l == 0:
                            rhs = eo_bf[:, kk, tr0:tr0 + C, :]
                        elif kk < 2:  # fwd outputs of L0: slot t+1
                            rhs = e_hist[0][:, tr0 + 1:tr0 + C + 1, kk, :]
                        else:         # bwd outputs of L0: slot t
                            rhs = e_hist[1][:, tr0:tr0 + C, kk - 2, :]
                        srcs.append((
                            (lambda kk_: lambda j, _:
                             ew_ih[:, sc["wih_base"] + kk_ * 8 + j, :])(kk),
                            rhs))
                    emit_xtilde(
                        srcs,
                        lambda j: ew_b[:, sc["b_base"] + j, :],
                        lambda j: sc["xsb"][:, j, :, :],
                        copy_eng=["s", "v"])

                def enc_gates(sc, k, i, u):
                    """matmuls + sigma/tanh for one scan, one step."""
                    lt = i * 4 + u if sc["fwd"] else (C - 1) - (i * 4 + u)
                    gp = psg_e.tile([128, 8, NL], FP, tag=f"g_{sc['tag'][-1]}",
                                    bufs=3)
                    nc.tensor.matmul(gp, ident,
                                     sc["xsb"][:, :, ds(lt, 1), :],
                                     start=True, stop=False)
                    hr = sc["hp"][u % 2]
                    wb = sc["whh_base"]
                    for kk in range(2):
                        for j in range(8):
                            nc.tensor.matmul(
                                gp[:, j, :], ew_hh[:, wb + kk * 8 + j, :],
                                hr[:, kk, :], start=False,
                                stop=(kk == 1 and j == 7))
                    sifo = stash.tile([128, 6, NL], FP, tag=f"sf_{sc['tag']}")
                    tg = stash.tile([128, 2, NL], FP, tag=f"tg_{sc['tag']}")
                    nc.scalar.activation(sifo, gp[:, 0:6, :], A.Sigmoid)
                    nc.scalar.activation(tg, gp[:, 6:8, :], A.Tanh)
                    sc["_sifo"], sc["_tg"] = sifo, tg

                def enc_cupd(sc):
                    sifo, tg = sc["_sifo"], sc["_tg"]
                    t1 = stash.tile([128, 2, NL], FP, tag=f"t1_{sc['tag']}")
                    t2 = stash.tile([128, 2, NL], FP, tag=f"t2_{sc['tag']}")
                    cst = sc["cst"]
                    nc.vector.tensor_mul(t1, sifo[:, 0:2, :], tg)
                    nc.vector.tensor_mul(t2, sifo[:, 2:4, :], cst)
                    nc.vector.tensor_add(cst, t1, t2)

                def enc_tcl(sc):
                    tcl = stash.tile([128, 2, NL], FP, tag=f"tc_{sc['tag']}")
                    nc.scalar.activation(tcl, sc["cst"], A.Tanh)
                    sc["_tcl"] = tcl

                def enc_hout(sc, k, i, u):
                    hw = sc["hp"][1 - u % 2]
                    nc.vector.tensor_mul(hw, sc["_sifo"][:, 4:6, :], sc["_tcl"])
                    if sc["hist"] is not None:
                        t0 = k * C
                        if sc["fwd"]:
                            wslot = t0 + i * 4 + u + 1
                        else:
                            wslot = S - 1 - t0 - (i * 4 + u)
                        nc.gpsimd.tensor_copy(
                            sc["hist"][:, ds(wslot, 1), :, :], hw)

                for l in range(2):
                    scans = [escan[f"e{l}0"], escan[f"e{l}1"]]
                    for k in range(SC):
                        for d, sc in enumerate(scans):
                            enc_xtilde(sc, l, d, k)
                        with tc.For_i(0, C // 4, 1, staggered_reset=True) as i:
                            for u in range(4):
                                # fully sequential per scan: scan B's PE
                                # burst overlaps scan A's Act/DVE tail, so
                                # the two chains run half-step offset
                                for sc in scans:
                                    enc_gates(sc, k, i, u)
                                    enc_cupd(sc)
                                    enc_tcl(sc)
                                    enc_hout(sc, k, i, u)

                for li, tag in enumerate(["e00", "e01", "e10", "e11"]):
                    nc.vector.tensor_copy(hinit[li], escan[tag]["hp"][0])
                    nc.vector.tensor_copy(cinit[li], escan[tag]["cst"])

            # =======================================================
            # DECODER (4-layer chunk-lagged wavefront, tails merged in
            # groups of 2 layers)
            # =======================================================
            with ExitStack() as dctx:
                dpool = dctx.enter_context(tc.tile_pool(name="dec", bufs=1))
                psg_d = dctx.enter_context(
                    tc.tile_pool(name="psgd", bufs=2, space="PSUM"))
                dw_ih0 = dpool.tile([97, 8, 128], BF)
                dw_ih = dpool.tile([128, 48, 128], BF)
                dw_b = dpool.tile([1, 24, 128], BF)
                dw_hh = dpool.tile([128, 64, 128], BF)
                fw = dpool.tile([128, 2, NMEL], BF)
                fb = dpool.tile([1, NMEL], BF)
                nc.sync.dma_start(out=dw_ih0, in_=d_dwih0)
                nc.sync.dma_start(out=dw_ih, in_=d_dwih)
                nc.sync.dma_start(out=dw_b, in_=d_db)
                nc.sync.dma_start(out=dw_hh, in_=d_dwhh)
                nc.sync.dma_start(out=fw, in_=d_fcw)
                nc.sync.dma_start(out=fb, in_=d_fcb)

                teach = dpool.tile([97, CB], BF)
                nc.vector.memset(teach, 0.0)
                nc.vector.memset(teach[96:97, :], 1.0)  # bias row
                mst = dpool.tile([NMEL, C, NL], FP)
                icst = dpool.tile([NMEL, 1], FP)

                # shared state tiles across the 4 layer-scans
                hseq = dpool.tile([128, 4, TD + 1, 2, NL], BF, name="hseq_u")
                cst_d = dpool.tile([128, 4, 2, NL], FP, name="cst_d")
                hp_d = [dpool.tile([128, 4, 2, NL], BF, name=f"hpd{u}")
                        for u in range(2)]
                # (layer, j) fused so the shared ident matmul's rhs AP has
                # the same 3-free-dim shape as the per-scan case
                xsb_u = dpool.tile([128, 4 * 8, C, NL], BF, name="xsb_u")
                for l in range(4):
                    nc.vector.tensor_copy(cst_d[:, l], cinit[l])
                    nc.vector.tensor_copy(hp_d[0][:, l], hinit[l])

                def fill_teacher(k):
                    t0 = k * C
                    if k == 0:
                        nc.sync.dma_start(
                            out=icst, in_=d_ictx.rearrange("(c o) -> c o", o=1))
                        nc.vector.tensor_copy(
                            mst[:, 0, :], icst.to_broadcast((NMEL, NL)))
                        nc.sync.dma_start(
                            out=mst[:, 1:, :], in_=d_mels[:, 0:C - 1, :])
                    else:
                        nc.sync.dma_start(
                            out=mst, in_=d_mels[:, t0 - 1:t0 + C - 1, :])
                    nc.vector.tensor_copy(teach[0:NMEL, :], mst)

                def dec_xtilde(l, k):
                    t0 = k * C
                    if l == 0:
                        emit_xtilde(
                            [(lambda j, _: dw_ih0[:, j, :], teach[:, :])],
                            None,
                            lambda j: xsb_u[:, j, :, :],
                            copy_eng=["s", "v"])
                    else:
                        srcs = []
                        for kk in range(2):
                            srcs.append((
                                (lambda kk_: lambda j, _:
                                 dw_ih[:, (l - 1) * 16 + kk_ * 8 + j, :])(kk),
                                hseq[:, l - 1, t0 + 1:t0 + C + 1, kk, :]))
                        emit_xtilde(
                            srcs,
                            lambda j: dw_b[:, (l - 1) * 8 + j, :],
                            lambda j: xsb_u[:, l * 8 + j, :, :],
                            copy_eng=["s", "v"])

                # one tail-group: layers glo..ghi (<=2), one step
                def dec_gates(glo, ghi, gi, i, u):
                    ng = ghi - glo + 1
                    lt = i * 4 + u
                    gp = psg_d.tile([128, 4, 8, NL], FP, tag=f"gd{gi}", bufs=3)
                    nc.tensor.matmul(
                        gp[:, 0:ng], ident,
                        xsb_u[:, glo * 8:(ghi + 1) * 8, ds(lt, 1), :],
                        start=True, stop=False)
                    for s in range(ng):
                        l = glo + s
                        hr = hp_d[u % 2]
                        for kk in range(2):
                            for j in range(8):
                                nc.tensor.matmul(
                                    gp[:, s, j, :],
                                    dw_hh[:, l * 16 + kk * 8 + j, :],
                                    hr[:, l, kk, :], start=False,
                                    stop=(s == ng - 1 and kk == 1 and j == 7))
                    sifo = stash.tile([128, 4, 6, NL], FP, tag=f"sfd{gi}")
                    tg = stash.tile([128, 4, 2, NL], FP, tag=f"tgd{gi}")
                    nc.scalar.activation(sifo[:, 0:ng], gp[:, 0:ng, 0:6, :],
                                         A.Sigmoid)
                    nc.scalar.activation(tg[:, 0:ng], gp[:, 0:ng, 6:8, :],
                                         A.Tanh)
                    return dict(glo=glo, ng=ng, gi=gi, sifo=sifo, tg=tg)

                def dec_cupd(g):
                    glo, ng, gi = g["glo"], g["ng"], g["gi"]
                    t1 = stash.tile([128, 4, 2, NL], FP, tag=f"t1d{gi}")
                    t2 = stash.tile([128, 4, 2, NL], FP, tag=f"t2d{gi}")
                    cs = cst_d[:, glo:glo + ng]
                    nc.vector.tensor_mul(t1[:, 0:ng], g["sifo"][:, 0:ng, 0:2, :],
                                         g["tg"][:, 0:ng])
                    nc.vector.tensor_mul(t2[:, 0:ng], g["sifo"][:, 0:ng, 2:4, :],
                                         cs)
                    nc.vector.tensor_add(cs, t1[:, 0:ng], t2[:, 0:ng])

                def dec_tcl(g):
                    glo, ng, gi = g["glo"], g["ng"], g["gi"]
                    tcl = stash.tile([128, 4, 2, NL], FP, tag=f"tcd{gi}")
                    nc.scalar.activation(tcl[:, 0:ng], cst_d[:, glo:glo + ng],
                                         A.Tanh)
                    g["tcl"] = tcl

                def dec_hout(g, p, i, u):
                    glo, ng = g["glo"], g["ng"]
                    hw = hp_d[1 - u % 2][:, glo:glo + ng]
                    nc.vector.tensor_mul(hw, g["sifo"][:, 0:ng, 4:6, :],
                                         g["tcl"][:, 0:ng])
                    for s in range(ng):
                        l = glo + s
                        wslot = (p - l) * C + i * 4 + u + 1
                        nc.gpsimd.tensor_copy(
                            hseq[:, l, ds(wslot, 1), :, :],
                            hp_d[1 - u % 2][:, l])

                def emit_fc(k):
                    t0 = k * C
                    fp = psum_x.tile([NMEL, CB], FP, tag="xp", name=f"fcp{k}")
                    for kk in range(2):
                        nc.tensor.matmul(
                            fp, fw[:, kk, :],
                            hseq[:, 3, t0 + 1:t0 + C + 1, kk, :],
                            start=(kk == 0), stop=False)
                    nc.tensor.matmul(fp, fb, ones, start=False, stop=True)
                    fst = stash.tile([NMEL, C, NL], FP, tag="fst",
                                     name=f"fst{k}")
                    nc.scalar.copy(fst, fp.rearrange("p (t b) -> p t b", b=NL))
                    if k == DC - 1:
                        nc.vector.memset(fst[:, C - 1:C, :], 0.0)
                    nc.sync.dma_start(out=d_out[:, t0:t0 + C, :], in_=fst)

                for p in range(DC + 3):
                    active = [l for l in range(4) if 0 <= p - l < DC]
                    lo, hi = min(active), max(active)
                    for l in active:
                        if l == 0:
                            fill_teacher(p)
                        dec_xtilde(l, p - l)
                    GSZ = 4
                    groups = []
                    g0 = lo
                    while g0 <= hi:
                        g1 = min(g0 + GSZ - 1, hi)
                        groups.append((g0, g1, (g0 - lo) // GSZ))
                        g0 = g1 + 1
                    with tc.For_i(0, C // 4, 1, staggered_reset=True) as i:
                        for u in range(4):
                            # sequential per group -> half-step offset chains
                            for (a, b, gi) in groups:
                                g = dec_gates(a, b, gi, i, u)
                                dec_cupd(g)
                                dec_tcl(g)
                                dec_hout(g, p, i, u)
                    if 0 <= p - 4 < DC:
                        emit_fc(p - 4)

                fc_done = {p - 4 for p in range(DC + 3) if 0 <= p - 4 < DC}
                for k in range(DC):
                    if k not in fc_done:
                        emit_fc(k)

    # pin act tables to the one set with sigmoid+tanh+copy so the compiler
    # never emits in-loop table reloads
    import concourse.bacc as _bacc_mod
    _orig = _bacc_mod.get_activation_tables

    def _pinned(arch):
        t = _orig(arch)
        need = {A.Sigmoid, A.Tanh, A.Copy, A.Identity}
        # preserve set order/length: ids are indices into act_info.json.
        # make every set that does NOT fully cover our funcs unusable for
        # them, so the chooser lands on one set for all activations.
        out = {}
        for k, v in t.items():
            if need <= v:
                out[k] = v
            else:
                out[k] = v - need
        return out if any(need <= v for v in out.values()) else t

    _bacc_mod.get_activation_tables = _pinned
    try:
        nc.compile()
    finally:
        _bacc_mod.get_activation_tables = _orig
    return nc


def _host_prep(inputs):
    """Slice batch across cores + pre-tile/cast weights (gate-permuted)."""
    bf16 = ml_dtypes.bfloat16

    def tiles_T(w, kchunks, jchunks):
        # w: [4H, D] fp32 -> [k, j] of w.T tiles [128, 128] bf16, j gate-permuted
        wT = np.ascontiguousarray(w.T).astype(bf16)  # [D, 4H]
        out = np.zeros((kchunks, jchunks, 128, 128), bf16)
        for k in range(kchunks):
            for j in range(jchunks):
                pj = GPERM[j] if jchunks == 8 else j
                out[k, j] = wT[k * 128:(k + 1) * 128, pj * 128:(pj + 1) * 128]
        return out

    enc_Wih = np.asarray(inputs["enc_Wih"], np.float32)
    enc_Whh = np.asarray(inputs["enc_Whh"], np.float32)
    enc_b = np.asarray(inputs["enc_b"], np.float32)
    dec_Wih0 = np.asarray(inputs["dec_Wih0"], np.float32)
    dec_Wih = np.asarray(inputs["dec_Wih"], np.float32)
    dec_Whh = np.asarray(inputs["dec_Whh"], np.float32)
    dec_b = np.asarray(inputs["dec_b"], np.float32)
    fc_W = np.asarray(inputs["fc_W"], np.float32)
    fc_b = np.asarray(inputs["fc_b"], np.float32)
    ictx = np.asarray(inputs["init_ctx"], np.float32).reshape(-1)

    ewih = np.zeros((2, 2, 4, 8, 128, 128), bf16)
    ewhh = np.zeros((2, 2, 2, 8, 128, 128), bf16)
    eb = np.zeros((2, 2, 8, 128), bf16)
    for l in range(2):
        for d in range(2):
            ewih[l, d] = tiles_T(enc_Wih[l, d], 4, 8)
            ewhh[l, d] = tiles_T(enc_Whh[l, d], 2, 8)
            eb[l, d] = enc_b[l, d].reshape(8, 128)[GPERM].astype(bf16)

    dwih0 = np.zeros((97, 8, 128), bf16)
    w0T = dec_Wih0.T.astype(bf16)  # [80, 1024]
    b0 = dec_b[0].reshape(8, 128)
    for j in range(8):
        pj = GPERM[j]
        dwih0[0:80, j] = w0T[:, pj * 128:(pj + 1) * 128]
        dwih0[96, j] = b0[pj].astype(bf16)

    dwih = np.zeros((3, 2, 8, 128, 128), bf16)
    db = np.zeros((3, 8, 128), bf16)
    for l in range(3):
        dwih[l] = tiles_T(dec_Wih[l], 2, 8)
        db[l] = dec_b[l + 1].reshape(8, 128)[GPERM].astype(bf16)
    dwhh = np.zeros((4, 2, 8, 128, 128), bf16)
    for l in range(4):
        dwhh[l] = tiles_T(dec_Whh[l], 2, 8)

    fcw = np.zeros((2, 128, NMEL), bf16)
    fWT = fc_W.T.astype(bf16)  # [256, 80]
    fcw[0] = fWT[0:128]
    fcw[1] = fWT[128:256]

    encout = np.asarray(inputs["encoder_outputs"], np.float32)
    mels = np.asarray(inputs["mels"], np.float32)
    N = encout.shape[0]
    nb = N // NCORES

    base = {
        "ictx": ictx,
        "ewih": np.ascontiguousarray(ewih.reshape(128, 128, 128).transpose(1, 0, 2)),
        "ewhh": np.ascontiguousarray(ewhh.reshape(64, 128, 128).transpose(1, 0, 2)),
        "eb": np.ascontiguousarray(eb.reshape(1, 32, 128)),
        "dwih0": dwih0,
        "dwih": np.ascontiguousarray(dwih.reshape(48, 128, 128).transpose(1, 0, 2)),
        "db": np.ascontiguousarray(db.reshape(1, 24, 128)),
        "dwhh": np.ascontiguousarray(dwhh.reshape(64, 128, 128).transpose(1, 0, 2)),
        "fcw": np.ascontiguousarray(fcw.transpose(1, 0, 2)),
        "fcb": fc_b.astype(bf16).reshape(1, NMEL),
    }
    S = encout.shape[1]
    in_maps = []
    for cid in range(NCORES):
        m = dict(base)
        eo = encout[cid * nb:(cid + 1) * nb]  # [nb, S, 512]
        m["encrhs"] = np.ascontiguousarray(
            eo.transpose(2, 1, 0).reshape(4, 128, S, nb).astype(bf16))
        # mels: [nb, NMEL, T] -> [NMEL, T, nb]
        m["mels"] = np.ascontiguousarray(
            mels[cid * nb:(cid + 1) * nb].transpose(1, 2, 0))
        in_maps.append(m)
    return in_maps


def kernel(encoder_outputs, mels, text_lengths, output_lengths,
           enc_Wih, enc_Whh, enc_b, dec_Wih0, dec_Wih, dec_Whh, dec_b,
           fc_W, fc_b, init_ctx):
    from concourse import bass_utils

    inputs = dict(encoder_outputs=encoder_outputs, mels=mels,
                  enc_Wih=enc_Wih, enc_Whh=enc_Whh, enc_b=enc_b,
                  dec_Wih0=dec_Wih0, dec_Wih=dec_Wih, dec_Whh=dec_Whh,
                  dec_b=dec_b, fc_W=fc_W, fc_b=fc_b, init_ctx=init_ctx)
    N, S, _ = np.asarray(encoder_outputs).shape
    T = np.asarray(mels).shape[2]
    key = (S, T)
    if key not in _prog_cache:
        _prog_cache[key] = _build_program(S, T)
    nc = _prog_cache[key]
    in_maps = _host_prep(inputs)
    res = bass_utils.run_bass_kernel_spmd(nc, in_maps, core_ids=list(range(NCORES)))
    nb = N // NCORES
    out = np.zeros((N, NMEL, T), np.float32)
    for cid in range(NCORES):
        # device out: [NMEL, T, nb] -> [nb, NMEL, T]
        out[cid * nb:(cid + 1) * nb] = res.results[cid]["out"].transpose(2, 0, 1)
    return (out,)


# revision 18
# speedup vs baseline: 1.0954x; 1.0954x over previous
"""Trainium2 Bass kernel for nn_Decoder (Tacotron-style LSTM encoder/decoder).

Architecture (8 NeuronCores, data-parallel over batch N=64 -> 8/core):
  - Transposed layout: hidden dim on SBUF partitions, (time,batch) on the
    free dim; input-side projections batched into large matmuls per 64-step
    chunk; only h @ Whh.T stays per-step.
  - Recurrent h state lives in ping-pong tiles with STATIC addresses (the
    2-step-unrolled For_i body alternates them), so the 16 per-step h-matmuls
    need no dynamic address computation on the PE sequencer.  The h history
    (consumed at chunk granularity by the next layer / FC) is written by the
    otherwise-idle GpSimd engine, off the recurrent critical path.
  - Gate order is host-permuted to [i,f,o,g] so one sigmoid covers i,f,o and
    one tanh covers g: 3 activation instructions per step instead of 4.
  - Activation tables are pinned to the single set containing sigmoid+tanh+
    copy, eliminating per-iteration act-table reloads.
  - Decoder runs a 4-layer chunk-lagged wavefront; the elementwise tails are
    merged across groups of 2 layers (one instruction covers both layers'
    gates), halving tail instruction count while keeping 2 independent
    dependency chains in flight.
  - Weights / h / x-projections in bf16, cell state c + PSUM in fp32.
"""

import numpy as np
import ml_dtypes

H = 256
NMEL = 80
D_ENC = 512
NCORES = 8
NL = 8          # batch per core
C = 64          # chunk (time) size

# new j-block order [i0,i1,f0,f1,o0,o1,g0,g1] <- old [i,i,f,f,g,g,o,o]
GPERM = [0, 1, 2, 3, 6, 7, 4, 5]

_prog_cache = {}


def _build_program(S, T):
    import concourse.bass as bass
    import concourse.mybir as mybir
    import concourse.tile as tile
    import concourse.bacc as bacc_mod
    from concourse import bacc
    from concourse.bass import ds
    from concourse.masks import make_identity
    from contextlib import ExitStack

    BF = mybir.dt.bfloat16
    FP = mybir.dt.float32
    A = mybir.ActivationFunctionType

    TD = T
    assert S % C == 0 and TD % C == 0
    SC = S // C
    DC = TD // C

    nc = bacc.Bacc("TRN2", target_bir_lowering=False, debug=False,
                   num_devices=NCORES)

    # ---------------- DRAM I/O ----------------
    d_encrhs = nc.dram_tensor("encrhs", [4, 128, S, NL], BF, kind="ExternalInput").ap()
    d_mels = nc.dram_tensor("mels", [NMEL, T, NL], FP, kind="ExternalInput").ap()
    d_ictx = nc.dram_tensor("ictx", [NMEL], FP, kind="ExternalInput").ap()
    d_ewih = nc.dram_tensor("ewih", [128, 128, 128], BF, kind="ExternalInput").ap()
    d_ewhh = nc.dram_tensor("ewhh", [128, 64, 128], BF, kind="ExternalInput").ap()
    d_eb = nc.dram_tensor("eb", [128, 32], FP, kind="ExternalInput").ap()
    d_dwih0 = nc.dram_tensor("dwih0", [97, 8, 128], BF, kind="ExternalInput").ap()
    d_dwih = nc.dram_tensor("dwih", [128, 48, 128], BF, kind="ExternalInput").ap()
    d_db = nc.dram_tensor("db", [128, 24], FP, kind="ExternalInput").ap()
    d_dwhh = nc.dram_tensor("dwhh", [128, 64, 128], BF, kind="ExternalInput").ap()
    d_fcw = nc.dram_tensor("fcw", [128, 2, NMEL], BF, kind="ExternalInput").ap()
    d_fcb = nc.dram_tensor("fcb", [NMEL, 1], FP, kind="ExternalInput").ap()
    d_out = nc.dram_tensor("out", [NMEL, T, NL], FP, kind="ExternalOutput").ap()

    CB = C * NL  # tokens per chunk = 512

    with tile.TileContext(nc) as tc:
        with ExitStack() as ctx:
            persist = ctx.enter_context(tc.tile_pool(name="persist", bufs=1))
            psum_x = ctx.enter_context(
                tc.tile_pool(name="psx", bufs=2, space="PSUM"))
            stash = ctx.enter_context(tc.tile_pool(name="stash", bufs=3))

            ident = persist.tile([128, 128], BF)
            make_identity(nc, ident)
            hinit = [persist.tile([128, 2, NL], BF, name=f"hinit{l}")
                     for l in range(4)]
            cinit = [persist.tile([128, 2, NL], FP, name=f"cinit{l}")
                     for l in range(4)]

            # ---------- batched input projection for one chunk ----------
            def emit_xtilde(xsrcs, brow, dst, copy_eng):
                """xsrcs: list of (lhsT_ap, rhs_fn) accumulated; brow:
                fn j -> [128,1] bias column folded into the psum->sbuf copy
                (saves an N=512 bias matmul per j); dst(j) -> [128, C*NL]."""
                nk = len(xsrcs)
                for j in range(8):
                    xp = psum_x.tile([128, CB], FP, tag="xp")
                    for kk, (wap, rhs) in enumerate(xsrcs):
                        nc.tensor.matmul(xp, wap(j, kk), rhs,
                                         start=(kk == 0), stop=(kk == nk - 1))
                    eng = copy_eng[j % len(copy_eng)]
                    if eng == "s":
                        if brow is not None:
                            nc.scalar.activation(dst(j), xp, A.Identity,
                                                 bias=brow(j))
                        else:
                            nc.scalar.copy(dst(j), xp)
                    else:
                        if brow is not None:
                            nc.vector.tensor_scalar_add(dst(j), xp, brow(j))
                        else:
                            nc.vector.tensor_copy(dst(j), xp)

            # =======================================================
            # ENCODER
            # =======================================================
            with ExitStack() as ectx:
                epool = ectx.enter_context(tc.tile_pool(name="enc", bufs=1))
                psg_e = ectx.enter_context(
                    tc.tile_pool(name="psge", bufs=2, space="PSUM"))
                ew_ih = epool.tile([128, 128, 128], BF)
                ew_hh = epool.tile([128, 64, 128], BF)
                ew_b = epool.tile([128, 32], FP)
                nc.sync.dma_start(out=ew_ih, in_=d_ewih)
                nc.sync.dma_start(out=ew_hh, in_=d_ewhh)
                nc.sync.dma_start(out=ew_b, in_=d_eb)

                eo_bf = epool.tile([128, 4, S, NL], BF)
                for kk in range(4):
                    nc.sync.dma_start(out=eo_bf[:, kk], in_=d_encrhs[kk])

                # history tiles only for layer 0 (consumed by layer 1)
                e_hist = [epool.tile([128, S + 1, 2, NL], BF, name=f"eh{d}")
                          for d in range(2)]

                escan = {}
                for (l, d) in [(0, 0), (0, 1), (1, 0), (1, 1)]:
                    tag = f"e{l}{d}"
                    si = l * 2 + d
                    cst = epool.tile([128, 2, NL], FP, name=f"c_{tag}")
                    hp = [epool.tile([128, 2, NL], BF, name=f"hp{u}_{tag}")
                          for u in range(2)]
                    xsb = epool.tile([128, 8, C, NL], BF, name=f"x_{tag}")
                    nc.vector.memset(cst, 0.0)
                    nc.vector.memset(hp[0], 0.0)
                    escan[tag] = dict(
                        tag=tag, fwd=(d == 0), cst=cst, hp=hp, xsb=xsb,
                        whh_base=si * 16, wih_base=si * 32, b_base=si * 8,
                        hist=e_hist[d] if l == 0 else None)

                def enc_xtilde(sc, l, d, k):
                    tr0 = k * C if d == 0 else S - (k + 1) * C
                    srcs = []
                    for kk in range(4):
                        if l == 0:
                            rhs = eo_bf[:, kk, tr0:tr0 + C, :]
                        elif kk < 2:  # fwd outputs of L0: slot t+1
                            rhs = e_hist[0][:, tr0 + 1:tr0 + C + 1, kk, :]
                        else:         # bwd outputs of L0: slot t
                            rhs = e_hist[1][:, tr0:tr0 + C, kk - 2, :]
                        srcs.append((
                            (lambda kk_: lambda j, _:
                             ew_ih[:, sc["wih_base"] + kk_ * 8 + j, :])(kk),
                            rhs))
                    emit_xtilde(
                        srcs,
                        lambda j, b=sc["b_base"]: ew_b[:, b + j:b + j + 1],
                        lambda j: sc["xsb"][:, j, :, :],
                        copy_eng=["s", "v"])

                def enc_gates(sc, k, i, u):
                    """matmuls + sigma/tanh for one scan, one step."""
                    lt = i * 8 + u if sc["fwd"] else (C - 1) - (i * 8 + u)
                    gp = psg_e.tile([128, 8, NL], FP, tag=f"g_{sc['tag'][-1]}",
                                    bufs=3)
                    nc.tensor.matmul(gp, ident,
                                     sc["xsb"][:, :, ds(lt, 1), :],
                                     start=True, stop=False)
                    hr = sc["hp"][u % 2]
                    wb = sc["whh_base"]
                    for kk in range(2):
                        for j in range(8):
                            nc.tensor.matmul(
                                gp[:, j, :], ew_hh[:, wb + kk * 8 + j, :],
                                hr[:, kk, :], start=False,
                                stop=(kk == 1 and j == 7))
                    sifo = stash.tile([128, 6, NL], FP, tag=f"sf_{sc['tag']}")
                    tg = stash.tile([128, 2, NL], FP, tag=f"tg_{sc['tag']}")
                    nc.scalar.activation(sifo, gp[:, 0:6, :], A.Sigmoid)
                    nc.scalar.activation(tg, gp[:, 6:8, :], A.Tanh)
                    sc["_sifo"], sc["_tg"] = sifo, tg

                def enc_cupd(sc):
                    sifo, tg = sc["_sifo"], sc["_tg"]
                    t1 = stash.tile([128, 2, NL], FP, tag=f"t1_{sc['tag']}")
                    t2 = stash.tile([128, 2, NL], FP, tag=f"t2_{sc['tag']}")
                    cst = sc["cst"]
                    nc.vector.tensor_mul(t1, sifo[:, 0:2, :], tg)
                    nc.vector.tensor_mul(t2, sifo[:, 2:4, :], cst)
                    nc.vector.tensor_add(cst, t1, t2)

                def enc_tcl(sc):
                    tcl = stash.tile([128, 2, NL], FP, tag=f"tc_{sc['tag']}")
                    nc.scalar.activation(tcl, sc["cst"], A.Tanh)
                    sc["_tcl"] = tcl

                def enc_hout(sc, k, i, u):
                    hw = sc["hp"][1 - u % 2]
                    nc.vector.tensor_mul(hw, sc["_sifo"][:, 4:6, :], sc["_tcl"])
                    if sc["hist"] is not None:
                        t0 = k * C
                        if sc["fwd"]:
                            wslot = t0 + i * 8 + u + 1
                        else:
                            wslot = S - 1 - t0 - (i * 8 + u)
                        nc.gpsimd.tensor_copy(
                            sc["hist"][:, ds(wslot, 1), :, :], hw)

                for l in range(2):
                    scans = [escan[f"e{l}0"], escan[f"e{l}1"]]
                    for k in range(SC):
                        for d, sc in enumerate(scans):
                            enc_xtilde(sc, l, d, k)
                        with tc.For_i(0, C // 8, 1, staggered_reset=True) as i:
                            for u in range(8):
                                # fully sequential per scan: scan B's PE
                                # burst overlaps scan A's Act/DVE tail, so
                                # the two chains run half-step offset
                                for sc in scans:
                                    enc_gates(sc, k, i, u)
                                    enc_cupd(sc)
                                    enc_tcl(sc)
                                    enc_hout(sc, k, i, u)

                for li, tag in enumerate(["e00", "e01", "e10", "e11"]):
                    nc.vector.tensor_copy(hinit[li], escan[tag]["hp"][0])
                    nc.vector.tensor_copy(cinit[li], escan[tag]["cst"])

            # =======================================================
            # DECODER (4-layer chunk-lagged wavefront, tails merged in
            # groups of 2 layers)
            # =======================================================
            with ExitStack() as dctx:
                dpool = dctx.enter_context(tc.tile_pool(name="dec", bufs=1))
                psg_d = dctx.enter_context(
                    tc.tile_pool(name="psgd", bufs=2, space="PSUM"))
                dw_ih0 = dpool.tile([97, 8, 128], BF)
                dw_ih = dpool.tile([128, 48, 128], BF)
                dw_b = dpool.tile([128, 24], FP)
                dw_hh = dpool.tile([128, 64, 128], BF)
                fw = dpool.tile([128, 2, NMEL], BF)
                fb = dpool.tile([NMEL, 1], FP)
                nc.sync.dma_start(out=dw_ih0, in_=d_dwih0)
                nc.sync.dma_start(out=dw_ih, in_=d_dwih)
                nc.sync.dma_start(out=dw_b, in_=d_db)
                nc.sync.dma_start(out=dw_hh, in_=d_dwhh)
                nc.sync.dma_start(out=fw, in_=d_fcw)
                nc.sync.dma_start(out=fb, in_=d_fcb)

                teach = dpool.tile([97, CB], BF)
                nc.vector.memset(teach, 0.0)
                nc.vector.memset(teach[96:97, :], 1.0)  # bias row
                mst2 = [dpool.tile([NMEL, C, NL], FP, name=f"mst{z}")
                        for z in range(2)]
                icst = dpool.tile([NMEL, 1], FP)

                # shared state tiles across the 4 layer-scans
                hseq = dpool.tile([128, 4, TD + 1, 2, NL], BF, name="hseq_u")
                cst_d = dpool.tile([128, 4, 2, NL], FP, name="cst_d")
                hp_d = [dpool.tile([128, 4, 2, NL], BF, name=f"hpd{u}")
                        for u in range(2)]
                # (layer, j) fused so the shared ident matmul's rhs AP has
                # the same 3-free-dim shape as the per-scan case
                xsb_u = dpool.tile([128, 4 * 8, C, NL], BF, name="xsb_u")
                for l in range(4):
                    nc.vector.tensor_copy(cst_d[:, l], cinit[l])
                    nc.vector.tensor_copy(hp_d[0][:, l], hinit[l])

                def fill_teacher(k):
                    cur = mst2[k % 2]
                    if k == 0:
                        nc.sync.dma_start(
                            out=icst, in_=d_ictx.rearrange("(c o) -> c o", o=1))
                        nc.vector.tensor_copy(
                            cur[:, 0, :], icst.to_broadcast((NMEL, NL)))
                        nc.sync.dma_start(
                            out=cur[:, 1:, :], in_=d_mels[:, 0:C - 1, :])
                    if k + 1 < DC:  # prefetch next chunk during this phase
                        nc.sync.dma_start(
                            out=mst2[(k + 1) % 2],
                            in_=d_mels[:, (k + 1) * C - 1:(k + 2) * C - 1, :])
                    nc.vector.tensor_copy(teach[0:NMEL, :], cur)

                def dec_xtilde(l, k):
                    t0 = k * C
                    if l == 0:
                        emit_xtilde(
                            [(lambda j, _: dw_ih0[:, j, :], teach[:, :])],
                            None,
                            lambda j: xsb_u[:, j, :, :],
                            copy_eng=["s", "v"])
                    else:
                        srcs = []
                        for kk in range(2):
                            srcs.append((
                                (lambda kk_: lambda j, _:
                                 dw_ih[:, (l - 1) * 16 + kk_ * 8 + j, :])(kk),
                                hseq[:, l - 1, t0 + 1:t0 + C + 1, kk, :]))
                        emit_xtilde(
                            srcs,
                            lambda j, b=(l - 1) * 8: dw_b[:, b + j:b + j + 1],
                            lambda j: xsb_u[:, l * 8 + j, :, :],
                            copy_eng=["s", "v"])

                # one tail-group: layers glo..ghi (<=2), one step
                def dec_gates(glo, ghi, gi, i, u):
                    ng = ghi - glo + 1
                    lt = i * 8 + u
                    gp = psg_d.tile([128, 4, 8, NL], FP, tag=f"gd{gi}", bufs=3)
                    nc.tensor.matmul(
                        gp[:, 0:ng], ident,
                        xsb_u[:, glo * 8:(ghi + 1) * 8, ds(lt, 1), :],
                        start=True, stop=False)
                    for s in range(ng):
                        l = glo + s
                        hr = hp_d[u % 2]
                        for kk in range(2):
                            for j in range(8):
                                nc.tensor.matmul(
                                    gp[:, s, j, :],
                                    dw_hh[:, l * 16 + kk * 8 + j, :],
                                    hr[:, l, kk, :], start=False,
                                    stop=(s == ng - 1 and kk == 1 and j == 7))
                    sifo = stash.tile([128, 4, 6, NL], FP, tag=f"sfd{gi}")
                    tg = stash.tile([128, 4, 2, NL], FP, tag=f"tgd{gi}")
                    nc.scalar.activation(sifo[:, 0:ng], gp[:, 0:ng, 0:6, :],
                                         A.Sigmoid)
                    nc.scalar.activation(tg[:, 0:ng], gp[:, 0:ng, 6:8, :],
                                         A.Tanh)
                    return dict(glo=glo, ng=ng, gi=gi, sifo=sifo, tg=tg)

                def dec_cupd(g):
                    glo, ng, gi = g["glo"], g["ng"], g["gi"]
                    t1 = stash.tile([128, 4, 2, NL], FP, tag=f"t1d{gi}")
                    t2 = stash.tile([128, 4, 2, NL], FP, tag=f"t2d{gi}")
                    cs = cst_d[:, glo:glo + ng]
                    nc.vector.tensor_mul(t1[:, 0:ng], g["sifo"][:, 0:ng, 0:2, :],
                                         g["tg"][:, 0:ng])
                    nc.vector.tensor_mul(t2[:, 0:ng], g["sifo"][:, 0:ng, 2:4, :],
                                         cs)
                    nc.vector.tensor_add(cs, t1[:, 0:ng], t2[:, 0:ng])

                def dec_tcl(g):
                    glo, ng, gi = g["glo"], g["ng"], g["gi"]
                    tcl = stash.tile([128, 4, 2, NL], FP, tag=f"tcd{gi}")
                    nc.scalar.activation(tcl[:, 0:ng], cst_d[:, glo:glo + ng],
                                         A.Tanh)
                    g["tcl"] = tcl

                def dec_hout(g, p, i, u):
                    glo, ng = g["glo"], g["ng"]
                    hw = hp_d[1 - u % 2][:, glo:glo + ng]
                    nc.vector.tensor_mul(hw, g["sifo"][:, 0:ng, 4:6, :],
                                         g["tcl"][:, 0:ng])
                    for s in range(ng):
                        l = glo + s
                        wslot = (p - l) * C + i * 8 + u + 1
                        nc.gpsimd.tensor_copy(
                            hseq[:, l, ds(wslot, 1), :, :],
                            hp_d[1 - u % 2][:, l])

                def emit_fc(k):
                    t0 = k * C
                    fp = psum_x.tile([NMEL, CB], FP, tag="xp", name=f"fcp{k}")
                    for kk in range(2):
                        nc.tensor.matmul(
                            fp, fw[:, kk, :],
                            hseq[:, 3, t0 + 1:t0 + C + 1, kk, :],
                            start=(kk == 0), stop=(kk == 1))
                    fst = stash.tile([NMEL, C, NL], FP, tag="fst",
                                     name=f"fst{k}")
                    nc.scalar.activation(fst, fp.rearrange("p (t b) -> p t b", b=NL),
                                         A.Identity, bias=fb)
                    if k == DC - 1:
                        nc.vector.memset(fst[:, C - 1:C, :], 0.0)
                    nc.sync.dma_start(out=d_out[:, t0:t0 + C, :], in_=fst)

                for p in range(DC + 3):
                    active = [l for l in range(4) if 0 <= p - l < DC]
                    lo, hi = min(active), max(active)
                    for l in active:
                        if l == 0:
                            fill_teacher(p)
                        dec_xtilde(l, p - l)
                    GSZ = 2
                    groups = []
                    g0 = lo
                    while g0 <= hi:
                        g1 = min(g0 + GSZ - 1, hi)
                        groups.append((g0, g1, (g0 - lo) // GSZ))
                        g0 = g1 + 1
                    with tc.For_i(0, C // 8, 1, staggered_reset=True) as i:
                        for u in range(8):
                            # sequential per group -> half-step offset chains
                            for (a, b, gi) in groups:
                                g = dec_gates(a, b, gi, i, u)
                                dec_cupd(g)
                                dec_tcl(g)
                                dec_hout(g, p, i, u)
                    if 0 <= p - 4 < DC:
                        emit_fc(p - 4)

                fc_done = {p - 4 for p in range(DC + 3) if 0 <= p - 4 < DC}
                for k in range(DC):
                    if k not in fc_done:
                        emit_fc(k)

    # pin act tables to the one set with sigmoid+tanh+copy so the compiler
    # never emits in-loop table reloads
    import concourse.bacc as _bacc_mod
    _orig = _bacc_mod.get_activation_tables

    def _pinned(arch):
        t = _orig(arch)
        need = {A.Sigmoid, A.Tanh, A.Copy, A.Identity}
        # preserve set order/length: ids are indices into act_info.json.
        # make every set that does NOT fully cover our funcs unusable for
        # them, so the chooser lands on one set for all activations.
        out = {}
        for k, v in t.items():
            if need <= v:
                out[k] = v
            else:
                out[k] = v - need
        return out if any(need <= v for v in out.values()) else t

    _bacc_mod.get_activation_tables = _pinned
    try:
        nc.compile()
    finally:
        _bacc_mod.get_activation_tables = _orig
    return nc


def _host_prep(inputs):
    """Slice batch across cores + pre-tile/cast weights (gate-permuted)."""
    bf16 = ml_dtypes.bfloat16

    def tiles_T(w, kchunks, jchunks):
        # w: [4H, D] fp32 -> [k, j] of w.T tiles [128, 128] bf16, j gate-permuted
        wT = np.ascontiguousarray(w.T).astype(bf16)  # [D, 4H]
        out = np.zeros((kchunks, jchunks, 128, 128), bf16)
        for k in range(kchunks):
            for j in range(jchunks):
                pj = GPERM[j] if jchunks == 8 else j
                out[k, j] = wT[k * 128:(k + 1) * 128, pj * 128:(pj + 1) * 128]
        return out

    enc_Wih = np.asarray(inputs["enc_Wih"], np.float32)
    enc_Whh = np.asarray(inputs["enc_Whh"], np.float32)
    enc_b = np.asarray(inputs["enc_b"], np.float32)
    dec_Wih0 = np.asarray(inputs["dec_Wih0"], np.float32)
    dec_Wih = np.asarray(inputs["dec_Wih"], np.float32)
    dec_Whh = np.asarray(inputs["dec_Whh"], np.float32)
    dec_b = np.asarray(inputs["dec_b"], np.float32)
    fc_W = np.asarray(inputs["fc_W"], np.float32)
    fc_b = np.asarray(inputs["fc_b"], np.float32)
    ictx = np.asarray(inputs["init_ctx"], np.float32).reshape(-1)

    ewih = np.zeros((2, 2, 4, 8, 128, 128), bf16)
    ewhh = np.zeros((2, 2, 2, 8, 128, 128), bf16)
    eb = np.zeros((2, 2, 8, 128), np.float32)
    for l in range(2):
        for d in range(2):
            ewih[l, d] = tiles_T(enc_Wih[l, d], 4, 8)
            ewhh[l, d] = tiles_T(enc_Whh[l, d], 2, 8)
            eb[l, d] = enc_b[l, d].reshape(8, 128)[GPERM]

    dwih0 = np.zeros((97, 8, 128), bf16)
    w0T = dec_Wih0.T.astype(bf16)  # [80, 1024]
    b0 = dec_b[0].reshape(8, 128)
    for j in range(8):
        pj = GPERM[j]
        dwih0[0:80, j] = w0T[:, pj * 128:(pj + 1) * 128]
        dwih0[96, j] = b0[pj].astype(bf16)

    dwih = np.zeros((3, 2, 8, 128, 128), bf16)
    db = np.zeros((3, 8, 128), np.float32)
    for l in range(3):
        dwih[l] = tiles_T(dec_Wih[l], 2, 8)
        db[l] = dec_b[l + 1].reshape(8, 128)[GPERM]
    dwhh = np.zeros((4, 2, 8, 128, 128), bf16)
    for l in range(4):
        dwhh[l] = tiles_T(dec_Whh[l], 2, 8)

    fcw = np.zeros((2, 128, NMEL), bf16)
    fWT = fc_W.T.astype(bf16)  # [256, 80]
    fcw[0] = fWT[0:128]
    fcw[1] = fWT[128:256]

    encout = np.asarray(inputs["encoder_outputs"], np.float32)
    mels = np.asarray(inputs["mels"], np.float32)
    N = encout.shape[0]
    nb = N // NCORES

    base = {
        "ictx": ictx,
        "ewih": np.ascontiguousarray(ewih.reshape(128, 128, 128).transpose(1, 0, 2)),
        "ewhh": np.ascontiguousarray(ewhh.reshape(64, 128, 128).transpose(1, 0, 2)),
        "eb": np.ascontiguousarray(eb.reshape(32, 128).T),
        "dwih0": dwih0,
        "dwih": np.ascontiguousarray(dwih.reshape(48, 128, 128).transpose(1, 0, 2)),
        "db": np.ascontiguousarray(db.reshape(24, 128).T),
        "dwhh": np.ascontiguousarray(dwhh.reshape(64, 128, 128).transpose(1, 0, 2)),
        "fcw": np.ascontiguousarray(fcw.transpose(1, 0, 2)),
        "fcb": fc_b.astype(np.float32).reshape(NMEL, 1),
    }
    S = encout.shape[1]
    in_maps = []
    for cid in range(NCORES):
        m = dict(base)
        eo = encout[cid * nb:(cid + 1) * nb]  # [nb, S, 512]
        m["encrhs"] = np.ascontiguousarray(
            eo.transpose(2, 1, 0).reshape(4, 128, S, nb).astype(bf16))
        # mels: [nb, NMEL, T] -> [NMEL, T, nb]
        m["mels"] = np.ascontiguousarray(
            mels[cid * nb:(cid + 1) * nb].transpose(1, 2, 0))
        in_maps.append(m)
    return in_maps


def kernel(encoder_outputs, mels, text_lengths, output_lengths,
           enc_Wih, enc_Whh, enc_b, dec_Wih0, dec_Wih, dec_Whh, dec_b,
           fc_W, fc_b, init_ctx):
    from concourse import bass_utils

    inputs = dict(encoder_outputs=encoder_outputs, mels=mels,
                  enc_Wih=enc_Wih, enc_Whh=enc_Whh, enc_b=enc_b,
                  dec_Wih0=dec_Wih0, dec_Wih=dec_Wih, dec_Whh=dec_Whh,
                  dec_b=dec_b, fc_W=fc_W, fc_b=fc_b, init_ctx=init_ctx)
    N, S, _ = np.asarray(encoder_outputs).shape
    T = np.asarray(mels).shape[2]
    key = (S, T)
    if key not in _prog_cache:
        _prog_cache[key] = _build_program(S, T)
    nc = _prog_cache[key]
    in_maps = _host_prep(inputs)
    res = bass_utils.run_bass_kernel_spmd(nc, in_maps, core_ids=list(range(NCORES)))
    nb = N // NCORES
    out = np.zeros((N, NMEL, T), np.float32)
    for cid in range(NCORES):
        # device out: [NMEL, T, nb] -> [nb, NMEL, T]
        out[cid * nb:(cid + 1) * nb] = res.results[cid]["out"].transpose(2, 0, 1)
    return (out,)


# revision 21
# speedup vs baseline: 1.1677x; 1.0660x over previous
"""Trainium2 Bass kernel for nn_Decoder (Tacotron-style LSTM encoder/decoder).

Architecture (8 NeuronCores, data-parallel over batch N=64 -> 8/core):
  - Transposed layout: hidden dim on SBUF partitions, (time,batch) on the
    free dim; input-side projections batched into large matmuls per 64-step
    chunk; only h @ Whh.T stays per-step.
  - Recurrent h state lives in ping-pong tiles with STATIC addresses (the
    2-step-unrolled For_i body alternates them), so the 16 per-step h-matmuls
    need no dynamic address computation on the PE sequencer.  The h history
    (consumed at chunk granularity by the next layer / FC) is written by the
    otherwise-idle GpSimd engine, off the recurrent critical path.
  - Gate order is host-permuted to [i,f,o,g] so one sigmoid covers i,f,o and
    one tanh covers g: 3 activation instructions per step instead of 4.
  - Activation tables are pinned to the single set containing sigmoid+tanh+
    copy, eliminating per-iteration act-table reloads.
  - Decoder runs a 4-layer chunk-lagged wavefront; the elementwise tails are
    merged across groups of 2 layers (one instruction covers both layers'
    gates), halving tail instruction count while keeping 2 independent
    dependency chains in flight.
  - Weights / h / x-projections in bf16, cell state c + PSUM in fp32.
"""

import numpy as np
import ml_dtypes

H = 256
NMEL = 80
D_ENC = 512
NCORES = 8
NL = 8          # batch per core
C = 64          # chunk (time) size

# new j-block order [i0,i1,f0,f1,o0,o1,g0,g1] <- old [i,i,f,f,g,g,o,o]
GPERM = [0, 1, 2, 3, 6, 7, 4, 5]

_prog_cache = {}


def _build_program(S, T):
    import concourse.bass as bass
    import concourse.mybir as mybir
    import concourse.tile as tile
    import concourse.bacc as bacc_mod
    from concourse import bacc
    from concourse.bass import ds
    from concourse.masks import make_identity
    from contextlib import ExitStack

    BF = mybir.dt.bfloat16
    FP = mybir.dt.float32
    A = mybir.ActivationFunctionType

    TD = T
    assert S % C == 0 and TD % C == 0
    SC = S // C
    DC = TD // C

    nc = bacc.Bacc("TRN2", target_bir_lowering=False, debug=False,
                   num_devices=NCORES)

    # ---------------- DRAM I/O ----------------
    d_encrhs = nc.dram_tensor("encrhs", [4, 128, S, NL], BF, kind="ExternalInput").ap()
    d_mels = nc.dram_tensor("mels", [NMEL, T, NL], FP, kind="ExternalInput").ap()
    d_ictx = nc.dram_tensor("ictx", [NMEL], FP, kind="ExternalInput").ap()
    d_ewih = nc.dram_tensor("ewih", [128, 128, 128], BF, kind="ExternalInput").ap()
    d_ewhh = nc.dram_tensor("ewhh", [128, 64, 128], BF, kind="ExternalInput").ap()
    d_eb = nc.dram_tensor("eb", [128, 32], FP, kind="ExternalInput").ap()
    d_dwih0 = nc.dram_tensor("dwih0", [97, 8, 128], BF, kind="ExternalInput").ap()
    d_dwih = nc.dram_tensor("dwih", [128, 48, 128], BF, kind="ExternalInput").ap()
    d_db = nc.dram_tensor("db", [128, 24], FP, kind="ExternalInput").ap()
    d_dwhh = nc.dram_tensor("dwhh", [128, 64, 128], BF, kind="ExternalInput").ap()
    d_fcw = nc.dram_tensor("fcw", [128, 2, NMEL], BF, kind="ExternalInput").ap()
    d_fcb = nc.dram_tensor("fcb", [NMEL, 1], FP, kind="ExternalInput").ap()
    d_out = nc.dram_tensor("out", [NMEL, T, NL], FP, kind="ExternalOutput").ap()

    CB = C * NL  # tokens per chunk = 512

    with tile.TileContext(nc) as tc:
        with ExitStack() as ctx:
            persist = ctx.enter_context(tc.tile_pool(name="persist", bufs=1))
            psum_x = ctx.enter_context(
                tc.tile_pool(name="psx", bufs=2, space="PSUM"))
            stash = ctx.enter_context(tc.tile_pool(name="stash", bufs=3))

            ident = persist.tile([128, 128], BF)
            make_identity(nc, ident)
            hinit = [persist.tile([128, 2, NL], BF, name=f"hinit{l}")
                     for l in range(4)]
            cinit = [persist.tile([128, 2, NL], FP, name=f"cinit{l}")
                     for l in range(4)]

            # ---------- batched input projection for one chunk ----------
            def emit_xtilde(xsrcs, brow, dst, copy_eng):
                """xsrcs: list of (lhsT_ap, rhs_fn) accumulated; brow:
                fn j -> [128,1] bias column folded into the psum->sbuf copy
                (saves an N=512 bias matmul per j); dst(j) -> [128, C*NL]."""
                nk = len(xsrcs)
                for j in range(8):
                    xp = psum_x.tile([128, CB], FP, tag="xp")
                    for kk, (wap, rhs) in enumerate(xsrcs):
                        nc.tensor.matmul(xp, wap(j, kk), rhs,
                                         start=(kk == 0), stop=(kk == nk - 1))
                    eng = copy_eng[j % len(copy_eng)]
                    if eng == "s":
                        if brow is not None:
                            nc.scalar.activation(dst(j), xp, A.Identity,
                                                 bias=brow(j))
                        else:
                            nc.scalar.copy(dst(j), xp)
                    else:
                        e = nc.vector if eng == "v" else nc.gpsimd
                        if brow is not None:
                            e.tensor_scalar_add(dst(j), xp, brow(j))
                        else:
                            e.tensor_copy(dst(j), xp)

            # =======================================================
            # ENCODER
            # =======================================================
            with ExitStack() as ectx:
                epool = ectx.enter_context(tc.tile_pool(name="enc", bufs=1))
                psg_e = ectx.enter_context(
                    tc.tile_pool(name="psge", bufs=2, space="PSUM"))
                ew_ih = epool.tile([128, 128, 128], BF)
                ew_hh = epool.tile([128, 64, 128], BF)
                ew_b = epool.tile([128, 32], FP)
                nc.sync.dma_start(out=ew_ih, in_=d_ewih)
                nc.sync.dma_start(out=ew_hh, in_=d_ewhh)
                nc.sync.dma_start(out=ew_b, in_=d_eb)

                eo_bf = epool.tile([128, 4, S, NL], BF)
                for kk in range(4):
                    nc.sync.dma_start(out=eo_bf[:, kk], in_=d_encrhs[kk])

                # history tiles only for layer 0 (consumed by layer 1)
                e_hist = [epool.tile([128, S + 1, 2, NL], BF, name=f"eh{d}")
                          for d in range(2)]

                escan = {}
                for (l, d) in [(0, 0), (0, 1), (1, 0), (1, 1)]:
                    tag = f"e{l}{d}"
                    si = l * 2 + d
                    cst = epool.tile([128, 2, NL], FP, name=f"c_{tag}")
                    hp = [epool.tile([128, 2, NL], BF, name=f"hp{u}_{tag}")
                          for u in range(2)]
                    xsb = epool.tile([128, 8, C, NL], BF, name=f"x_{tag}")
                    nc.vector.memset(cst, 0.0)
                    nc.vector.memset(hp[0], 0.0)
                    escan[tag] = dict(
                        tag=tag, fwd=(d == 0), cst=cst, hp=hp, xsb=xsb,
                        whh_base=si * 16, wih_base=si * 32, b_base=si * 8,
                        hist=e_hist[d] if l == 0 else None)

                def enc_xtilde(sc, l, d, k):
                    tr0 = k * C if d == 0 else S - (k + 1) * C
                    srcs = []
                    for kk in range(4):
                        if l == 0:
                            rhs = eo_bf[:, kk, tr0:tr0 + C, :]
                        elif kk < 2:  # fwd outputs of L0: slot t+1
                            rhs = e_hist[0][:, tr0 + 1:tr0 + C + 1, kk, :]
                        else:         # bwd outputs of L0: slot t
                            rhs = e_hist[1][:, tr0:tr0 + C, kk - 2, :]
                        srcs.append((
                            (lambda kk_: lambda j, _:
                             ew_ih[:, sc["wih_base"] + kk_ * 8 + j, :])(kk),
                            rhs))
                    emit_xtilde(
                        srcs,
                        lambda j, b=sc["b_base"]: ew_b[:, b + j:b + j + 1],
                        lambda j: sc["xsb"][:, j, :, :],
                        copy_eng=["s", "v"])

                def enc_gates(sc, k, i, u):
                    """matmuls + sigma/tanh for one scan, one step."""
                    lt = i * 8 + u if sc["fwd"] else (C - 1) - (i * 8 + u)
                    gp = psg_e.tile([128, 8, NL], FP, tag=f"g_{sc['tag'][-1]}",
                                    bufs=3)
                    nc.tensor.matmul(gp, ident,
                                     sc["xsb"][:, :, ds(lt, 1), :],
                                     start=True, stop=False)
                    hr = sc["hp"][u % 2]
                    wb = sc["whh_base"]
                    for kk in range(2):
                        for j in range(8):
                            nc.tensor.matmul(
                                gp[:, j, :], ew_hh[:, wb + kk * 8 + j, :],
                                hr[:, kk, :], start=False,
                                stop=(kk == 1 and j == 7))
                    sifo = stash.tile([128, 6, NL], FP, tag=f"sf_{sc['tag']}")
                    tg = stash.tile([128, 2, NL], FP, tag=f"tg_{sc['tag']}")
                    nc.scalar.activation(sifo, gp[:, 0:6, :], A.Sigmoid)
                    nc.scalar.activation(tg, gp[:, 6:8, :], A.Tanh)
                    sc["_sifo"], sc["_tg"] = sifo, tg

                def enc_cupd(sc):
                    sifo, tg = sc["_sifo"], sc["_tg"]
                    t1 = stash.tile([128, 2, NL], FP, tag=f"t1_{sc['tag']}")
                    t2 = stash.tile([128, 2, NL], FP, tag=f"t2_{sc['tag']}")
                    cst = sc["cst"]
                    nc.vector.tensor_mul(t1, sifo[:, 0:2, :], tg)
                    nc.vector.tensor_mul(t2, sifo[:, 2:4, :], cst)
                    nc.vector.tensor_add(cst, t1, t2)

                def enc_tcl(sc):
                    tcl = stash.tile([128, 2, NL], FP, tag=f"tc_{sc['tag']}")
                    nc.scalar.activation(tcl, sc["cst"], A.Tanh)
                    sc["_tcl"] = tcl

                def enc_hout(sc, k, i, u):
                    hw = sc["hp"][1 - u % 2]
                    nc.vector.tensor_mul(hw, sc["_sifo"][:, 4:6, :], sc["_tcl"])
                    if sc["hist"] is not None:
                        t0 = k * C
                        if sc["fwd"]:
                            wslot = t0 + i * 8 + u + 1
                        else:
                            wslot = S - 1 - t0 - (i * 8 + u)
                        nc.gpsimd.tensor_copy(
                            sc["hist"][:, ds(wslot, 1), :, :], hw)

                for l in range(2):
                    scans = [escan[f"e{l}0"], escan[f"e{l}1"]]
                    for k in range(SC):
                        for d, sc in enumerate(scans):
                            enc_xtilde(sc, l, d, k)
                        with tc.For_i(0, C // 8, 1, staggered_reset=False) as i:
                            for u in range(8):
                                # fully sequential per scan: scan B's PE
                                # burst overlaps scan A's Act/DVE tail, so
                                # the two chains run half-step offset
                                for sc in scans:
                                    enc_gates(sc, k, i, u)
                                    enc_cupd(sc)
                                    enc_tcl(sc)
                                    enc_hout(sc, k, i, u)

                for li, tag in enumerate(["e00", "e01", "e10", "e11"]):
                    nc.vector.tensor_copy(hinit[li], escan[tag]["hp"][0])
                    nc.vector.tensor_copy(cinit[li], escan[tag]["cst"])

            # =======================================================
            # DECODER (4-layer chunk-lagged wavefront, tails merged in
            # groups of 2 layers)
            # =======================================================
            with ExitStack() as dctx:
                dpool = dctx.enter_context(tc.tile_pool(name="dec", bufs=1))
                psg_d = dctx.enter_context(
                    tc.tile_pool(name="psgd", bufs=2, space="PSUM"))
                dw_ih0 = dpool.tile([97, 8, 128], BF)
                dw_ih = dpool.tile([128, 48, 128], BF)
                dw_b = dpool.tile([128, 24], FP)
                dw_hh = dpool.tile([128, 64, 128], BF)
                fw = dpool.tile([128, 2, NMEL], BF)
                fb = dpool.tile([NMEL, 1], FP)
                nc.sync.dma_start(out=dw_ih0, in_=d_dwih0)
                nc.sync.dma_start(out=dw_ih, in_=d_dwih)
                nc.sync.dma_start(out=dw_b, in_=d_db)
                nc.sync.dma_start(out=dw_hh, in_=d_dwhh)
                nc.sync.dma_start(out=fw, in_=d_fcw)
                nc.sync.dma_start(out=fb, in_=d_fcb)

                teach = dpool.tile([97, CB], BF)
                nc.vector.memset(teach, 0.0)
                nc.vector.memset(teach[96:97, :], 1.0)  # bias row
                mst2 = [dpool.tile([NMEL, C, NL], FP, name=f"mst{z}")
                        for z in range(2)]
                icst = dpool.tile([NMEL, 1], FP)

                # shared state tiles across the 4 layer-scans
                hseq = dpool.tile([128, 4, TD + 1, 2, NL], BF, name="hseq_u")
                cst_d = dpool.tile([128, 4, 2, NL], FP, name="cst_d")
                hp_d = [dpool.tile([128, 4, 2, NL], BF, name=f"hpd{u}")
                        for u in range(2)]
                # (layer, j) fused so the shared ident matmul's rhs AP has
                # the same 3-free-dim shape as the per-scan case
                xsb_u = dpool.tile([128, 4 * 8, C, NL], BF, name="xsb_u")
                for l in range(4):
                    nc.vector.tensor_copy(cst_d[:, l], cinit[l])
                    nc.vector.tensor_copy(hp_d[0][:, l], hinit[l])

                def fill_teacher(k):
                    cur = mst2[k % 2]
                    if k == 0:
                        nc.sync.dma_start(
                            out=icst, in_=d_ictx.rearrange("(c o) -> c o", o=1))
                        nc.vector.tensor_copy(
                            cur[:, 0, :], icst.to_broadcast((NMEL, NL)))
                        nc.sync.dma_start(
                            out=cur[:, 1:, :], in_=d_mels[:, 0:C - 1, :])
                    if k + 1 < DC:  # prefetch next chunk during this phase
                        nc.sync.dma_start(
                            out=mst2[(k + 1) % 2],
                            in_=d_mels[:, (k + 1) * C - 1:(k + 2) * C - 1, :])
                    nc.vector.tensor_copy(teach[0:NMEL, :], cur)

                def dec_xtilde(l, k):
                    t0 = k * C
                    if l == 0:
                        emit_xtilde(
                            [(lambda j, _: dw_ih0[:, j, :], teach[:, :])],
                            None,
                            lambda j: xsb_u[:, j, :, :],
                            copy_eng=["s", "v"])
                    else:
                        srcs = []
                        for kk in range(2):
                            srcs.append((
                                (lambda kk_: lambda j, _:
                                 dw_ih[:, (l - 1) * 16 + kk_ * 8 + j, :])(kk),
                                hseq[:, l - 1, t0 + 1:t0 + C + 1, kk, :]))
                        emit_xtilde(
                            srcs,
                            lambda j, b=(l - 1) * 8: dw_b[:, b + j:b + j + 1],
                            lambda j: xsb_u[:, l * 8 + j, :, :],
                            copy_eng=["s", "v"])

                # one tail-group: layers glo..ghi (<=2), one step
                def dec_gates(glo, ghi, gi, i, u):
                    ng = ghi - glo + 1
                    lt = i * 8 + u
                    gp = psg_d.tile([128, 4, 8, NL], FP, tag=f"gd{gi}", bufs=3)
                    nc.tensor.matmul(
                        gp[:, 0:ng], ident,
                        xsb_u[:, glo * 8:(ghi + 1) * 8, ds(lt, 1), :],
                        start=True, stop=False)
                    for s in range(ng):
                        l = glo + s
                        hr = hp_d[u % 2]
                        for kk in range(2):
                            for j in range(8):
                                nc.tensor.matmul(
                                    gp[:, s, j, :],
                                    dw_hh[:, l * 16 + kk * 8 + j, :],
                                    hr[:, l, kk, :], start=False,
                                    stop=(s == ng - 1 and kk == 1 and j == 7))
                    sifo = stash.tile([128, 4, 6, NL], FP, tag=f"sfd{gi}")
                    tg = stash.tile([128, 4, 2, NL], FP, tag=f"tgd{gi}")
                    nc.scalar.activation(sifo[:, 0:ng], gp[:, 0:ng, 0:6, :],
                                         A.Sigmoid)
                    nc.scalar.activation(tg[:, 0:ng], gp[:, 0:ng, 6:8, :],
                                         A.Tanh)
                    return dict(glo=glo, ng=ng, gi=gi, sifo=sifo, tg=tg)

                def dec_cupd(g):
                    glo, ng, gi = g["glo"], g["ng"], g["gi"]
                    t1 = stash.tile([128, 4, 2, NL], FP, tag=f"t1d{gi}")
                    t2 = stash.tile([128, 4, 2, NL], FP, tag=f"t2d{gi}")
                    cs = cst_d[:, glo:glo + ng]
                    nc.vector.tensor_mul(t1[:, 0:ng], g["sifo"][:, 0:ng, 0:2, :],
                                         g["tg"][:, 0:ng])
                    nc.vector.tensor_mul(t2[:, 0:ng], g["sifo"][:, 0:ng, 2:4, :],
                                         cs)
                    nc.vector.tensor_add(cs, t1[:, 0:ng], t2[:, 0:ng])

                def dec_tcl(g):
                    glo, ng, gi = g["glo"], g["ng"], g["gi"]
                    tcl = stash.tile([128, 4, 2, NL], FP, tag=f"tcd{gi}")
                    nc.scalar.activation(tcl[:, 0:ng], cst_d[:, glo:glo + ng],
                                         A.Tanh)
                    g["tcl"] = tcl

                def dec_hout(g, p, i, u):
                    glo, ng = g["glo"], g["ng"]
                    hw = hp_d[1 - u % 2][:, glo:glo + ng]
                    nc.vector.tensor_mul(hw, g["sifo"][:, 0:ng, 4:6, :],
                                         g["tcl"][:, 0:ng])
                    for s in range(ng):
                        l = glo + s
                        wslot = (p - l) * C + i * 8 + u + 1
                        nc.gpsimd.tensor_copy(
                            hseq[:, l, ds(wslot, 1), :, :],
                            hp_d[1 - u % 2][:, l])

                def emit_fc(k):
                    t0 = k * C
                    fp = psum_x.tile([NMEL, CB], FP, tag="xp", name=f"fcp{k}")
                    for kk in range(2):
                        nc.tensor.matmul(
                            fp, fw[:, kk, :],
                            hseq[:, 3, t0 + 1:t0 + C + 1, kk, :],
                            start=(kk == 0), stop=(kk == 1))
                    fst = stash.tile([NMEL, C, NL], FP, tag="fst",
                                     name=f"fst{k}")
                    nc.scalar.activation(fst, fp.rearrange("p (t b) -> p t b", b=NL),
                                         A.Identity, bias=fb)
                    if k == DC - 1:
                        nc.vector.memset(fst[:, C - 1:C, :], 0.0)
                    nc.sync.dma_start(out=d_out[:, t0:t0 + C, :], in_=fst)

                for p in range(DC + 3):
                    active = [l for l in range(4) if 0 <= p - l < DC]
                    lo, hi = min(active), max(active)
                    for l in active:
                        if l == 0:
                            fill_teacher(p)
                        dec_xtilde(l, p - l)
                    GSZ = 2
                    groups = []
                    g0 = lo
                    while g0 <= hi:
                        g1 = min(g0 + GSZ - 1, hi)
                        groups.append((g0, g1, (g0 - lo) // GSZ))
                        g0 = g1 + 1
                    with tc.For_i(0, C // 8, 1, staggered_reset=False) as i:
                        for u in range(8):
                            # sequential per group -> half-step offset chains
                            for (a, b, gi) in groups:
                                g = dec_gates(a, b, gi, i, u)
                                dec_cupd(g)
                                dec_tcl(g)
                                dec_hout(g, p, i, u)
                    if 0 <= p - 4 < DC:
                        emit_fc(p - 4)

                fc_done = {p - 4 for p in range(DC + 3) if 0 <= p - 4 < DC}
                for k in range(DC):
                    if k not in fc_done:
                        emit_fc(k)

    # pin act tables to the one set with sigmoid+tanh+copy so the compiler
    # never emits in-loop table reloads
    import concourse.bacc as _bacc_mod
    _orig = _bacc_mod.get_activation_tables

    def _pinned(arch):
        t = _orig(arch)
        need = {A.Sigmoid, A.Tanh, A.Copy, A.Identity}
        # preserve set order/length: ids are indices into act_info.json.
        # make every set that does NOT fully cover our funcs unusable for
        # them, so the chooser lands on one set for all activations.
        out = {}
        for k, v in t.items():
            if need <= v:
                out[k] = v
            else:
                out[k] = v - need
        return out if any(need <= v for v in out.values()) else t

    _bacc_mod.get_activation_tables = _pinned
    try:
        nc.compile()
    finally:
        _bacc_mod.get_activation_tables = _orig
    return nc


def _host_prep(inputs):
    """Slice batch across cores + pre-tile/cast weights (gate-permuted)."""
    bf16 = ml_dtypes.bfloat16

    def tiles_T(w, kchunks, jchunks):
        # w: [4H, D] fp32 -> [k, j] of w.T tiles [128, 128] bf16, j gate-permuted
        wT = np.ascontiguousarray(w.T).astype(bf16)  # [D, 4H]
        out = np.zeros((kchunks, jchunks, 128, 128), bf16)
        for k in range(kchunks):
            for j in range(jchunks):
                pj = GPERM[j] if jchunks == 8 else j
                out[k, j] = wT[k * 128:(k + 1) * 128, pj * 128:(pj + 1) * 128]
        return out

    enc_Wih = np.asarray(inputs["enc_Wih"], np.float32)
    enc_Whh = np.asarray(inputs["enc_Whh"], np.float32)
    enc_b = np.asarray(inputs["enc_b"], np.float32)
    dec_Wih0 = np.asarray(inputs["dec_Wih0"], np.float32)
    dec_Wih = np.asarray(inputs["dec_Wih"], np.float32)
    dec_Whh = np.asarray(inputs["dec_Whh"], np.float32)
    dec_b = np.asarray(inputs["dec_b"], np.float32)
    fc_W = np.asarray(inputs["fc_W"], np.float32)
    fc_b = np.asarray(inputs["fc_b"], np.float32)
    ictx = np.asarray(inputs["init_ctx"], np.float32).reshape(-1)

    ewih = np.zeros((2, 2, 4, 8, 128, 128), bf16)
    ewhh = np.zeros((2, 2, 2, 8, 128, 128), bf16)
    eb = np.zeros((2, 2, 8, 128), np.float32)
    for l in range(2):
        for d in range(2):
            ewih[l, d] = tiles_T(enc_Wih[l, d], 4, 8)
            ewhh[l, d] = tiles_T(enc_Whh[l, d], 2, 8)
            eb[l, d] = enc_b[l, d].reshape(8, 128)[GPERM]

    dwih0 = np.zeros((97, 8, 128), bf16)
    w0T = dec_Wih0.T.astype(bf16)  # [80, 1024]
    b0 = dec_b[0].reshape(8, 128)
    for j in range(8):
        pj = GPERM[j]
        dwih0[0:80, j] = w0T[:, pj * 128:(pj + 1) * 128]
        dwih0[96, j] = b0[pj].astype(bf16)

    dwih = np.zeros((3, 2, 8, 128, 128), bf16)
    db = np.zeros((3, 8, 128), np.float32)
    for l in range(3):
        dwih[l] = tiles_T(dec_Wih[l], 2, 8)
        db[l] = dec_b[l + 1].reshape(8, 128)[GPERM]
    dwhh = np.zeros((4, 2, 8, 128, 128), bf16)
    for l in range(4):
        dwhh[l] = tiles_T(dec_Whh[l], 2, 8)

    fcw = np.zeros((2, 128, NMEL), bf16)
    fWT = fc_W.T.astype(bf16)  # [256, 80]
    fcw[0] = fWT[0:128]
    fcw[1] = fWT[128:256]

    encout = np.asarray(inputs["encoder_outputs"], np.float32)
    mels = np.asarray(inputs["mels"], np.float32)
    N = encout.shape[0]
    nb = N // NCORES

    base = {
        "ictx": ictx,
        "ewih": np.ascontiguousarray(ewih.reshape(128, 128, 128).transpose(1, 0, 2)),
        "ewhh": np.ascontiguousarray(ewhh.reshape(64, 128, 128).transpose(1, 0, 2)),
        "eb": np.ascontiguousarray(eb.reshape(32, 128).T),
        "dwih0": dwih0,
        "dwih": np.ascontiguousarray(dwih.reshape(48, 128, 128).transpose(1, 0, 2)),
        "db": np.ascontiguousarray(db.reshape(24, 128).T),
        "dwhh": np.ascontiguousarray(dwhh.reshape(64, 128, 128).transpose(1, 0, 2)),
        "fcw": np.ascontiguousarray(fcw.transpose(1, 0, 2)),
        "fcb": fc_b.astype(np.float32).reshape(NMEL, 1),
    }
    S = encout.shape[1]
    in_maps = []
    for cid in range(NCORES):
        m = dict(base)
        eo = encout[cid * nb:(cid + 1) * nb]  # [nb, S, 512]
        m["encrhs"] = np.ascontiguousarray(
            eo.transpose(2, 1, 0).reshape(4, 128, S, nb).astype(bf16))
        # mels: [nb, NMEL, T] -> [NMEL, T, nb]
        m["mels"] = np.ascontiguousarray(
            mels[cid * nb:(cid + 1) * nb].transpose(1, 2, 0))
        in_maps.append(m)
    return in_maps


def kernel(encoder_outputs, mels, text_lengths, output_lengths,
           enc_Wih, enc_Whh, enc_b, dec_Wih0, dec_Wih, dec_Whh, dec_b,
           fc_W, fc_b, init_ctx):
    from concourse import bass_utils

    inputs = dict(encoder_outputs=encoder_outputs, mels=mels,
                  enc_Wih=enc_Wih, enc_Whh=enc_Whh, enc_b=enc_b,
                  dec_Wih0=dec_Wih0, dec_Wih=dec_Wih, dec_Whh=dec_Whh,
                  dec_b=dec_b, fc_W=fc_W, fc_b=fc_b, init_ctx=init_ctx)
    N, S, _ = np.asarray(encoder_outputs).shape
    T = np.asarray(mels).shape[2]
    key = (S, T)
    if key not in _prog_cache:
        _prog_cache[key] = _build_program(S, T)
    nc = _prog_cache[key]
    in_maps = _host_prep(inputs)
    res = bass_utils.run_bass_kernel_spmd(nc, in_maps, core_ids=list(range(NCORES)))
    nb = N // NCORES
    out = np.zeros((N, NMEL, T), np.float32)
    for cid in range(NCORES):
        # device out: [NMEL, T, nb] -> [nb, NMEL, T]
        out[cid * nb:(cid + 1) * nb] = res.results[cid]["out"].transpose(2, 0, 1)
    return (out,)


# revision 22
# speedup vs baseline: 1.1997x; 1.0275x over previous
"""Trainium2 Bass kernel for nn_Decoder (Tacotron-style LSTM encoder/decoder).

Architecture (8 NeuronCores, data-parallel over batch N=64 -> 8/core):
  - Transposed layout: hidden dim on SBUF partitions, (time,batch) on the
    free dim; input-side projections batched into large matmuls per 64-step
    chunk; only h @ Whh.T stays per-step.
  - Recurrent h state lives in ping-pong tiles with STATIC addresses (the
    2-step-unrolled For_i body alternates them), so the 16 per-step h-matmuls
    need no dynamic address computation on the PE sequencer.  The h history
    (consumed at chunk granularity by the next layer / FC) is written by the
    otherwise-idle GpSimd engine, off the recurrent critical path.
  - Gate order is host-permuted to [i,f,o,g] so one sigmoid covers i,f,o and
    one tanh covers g: 3 activation instructions per step instead of 4.
  - Activation tables are pinned to the single set containing sigmoid+tanh+
    copy, eliminating per-iteration act-table reloads.
  - Decoder runs a 4-layer chunk-lagged wavefront; the elementwise tails are
    merged across groups of 2 layers (one instruction covers both layers'
    gates), halving tail instruction count while keeping 2 independent
    dependency chains in flight.
  - Weights / h / x-projections in bf16, cell state c + PSUM in fp32.
"""

import numpy as np
import ml_dtypes

H = 256
NMEL = 80
D_ENC = 512
NCORES = 8
NL = 8          # batch per core
C = 64          # chunk (time) size

# new j-block order [i0,i1,f0,f1,o0,o1,g0,g1] <- old [i,i,f,f,g,g,o,o]
GPERM = [0, 1, 2, 3, 6, 7, 4, 5]

_prog_cache = {}


def _build_program(S, T):
    import concourse.bass as bass
    import concourse.mybir as mybir
    import concourse.tile as tile
    import concourse.bacc as bacc_mod
    from concourse import bacc
    from concourse.bass import ds
    from concourse.masks import make_identity
    from contextlib import ExitStack

    BF = mybir.dt.bfloat16
    FP = mybir.dt.float32
    A = mybir.ActivationFunctionType

    TD = T
    assert S % C == 0 and TD % C == 0
    SC = S // C
    DC = TD // C

    nc = bacc.Bacc("TRN2", target_bir_lowering=False, debug=False,
                   num_devices=NCORES)

    # ---------------- DRAM I/O ----------------
    d_encrhs = nc.dram_tensor("encrhs", [4, 128, S, NL], BF, kind="ExternalInput").ap()
    d_mels = nc.dram_tensor("mels", [NMEL, T, NL], FP, kind="ExternalInput").ap()
    d_ictx = nc.dram_tensor("ictx", [NMEL], FP, kind="ExternalInput").ap()
    d_ewih = nc.dram_tensor("ewih", [128, 128, 128], BF, kind="ExternalInput").ap()
    d_ewhh = nc.dram_tensor("ewhh", [128, 64, 128], BF, kind="ExternalInput").ap()
    d_eb = nc.dram_tensor("eb", [128, 32], FP, kind="ExternalInput").ap()
    d_dwih0 = nc.dram_tensor("dwih0", [97, 8, 128], BF, kind="ExternalInput").ap()
    d_dwih = nc.dram_tensor("dwih", [128, 48, 128], BF, kind="ExternalInput").ap()
    d_db = nc.dram_tensor("db", [128, 24], FP, kind="ExternalInput").ap()
    d_dwhh = nc.dram_tensor("dwhh", [128, 64, 128], BF, kind="ExternalInput").ap()
    d_fcw = nc.dram_tensor("fcw", [128, 2, NMEL], BF, kind="ExternalInput").ap()
    d_fcb = nc.dram_tensor("fcb", [NMEL, 1], FP, kind="ExternalInput").ap()
    d_out = nc.dram_tensor("out", [NMEL, T, NL], FP, kind="ExternalOutput").ap()

    CB = C * NL  # tokens per chunk = 512

    with tile.TileContext(nc) as tc:
        with ExitStack() as ctx:
            persist = ctx.enter_context(tc.tile_pool(name="persist", bufs=1))
            psum_x = ctx.enter_context(
                tc.tile_pool(name="psx", bufs=2, space="PSUM"))
            stash = ctx.enter_context(tc.tile_pool(name="stash", bufs=3))

            ident = persist.tile([128, 128], BF)
            make_identity(nc, ident)
            hinit = [persist.tile([128, 2, NL], BF, name=f"hinit{l}")
                     for l in range(4)]
            cinit = [persist.tile([128, 2, NL], FP, name=f"cinit{l}")
                     for l in range(4)]

            # ---------- batched input projection for one chunk ----------
            def emit_xtilde(xsrcs, brow, dst, copy_eng):
                """xsrcs: list of (lhsT_ap, rhs_fn) accumulated; brow:
                fn j -> [128,1] bias column folded into the psum->sbuf copy
                (saves an N=512 bias matmul per j); dst(j) -> [128, C*NL]."""
                nk = len(xsrcs)
                for j in range(8):
                    xp = psum_x.tile([128, CB], FP, tag="xp")
                    for kk, (wap, rhs) in enumerate(xsrcs):
                        nc.tensor.matmul(xp, wap(j, kk), rhs,
                                         start=(kk == 0), stop=(kk == nk - 1))
                    eng = copy_eng[j % len(copy_eng)]
                    if eng == "s":
                        if brow is not None:
                            nc.scalar.activation(dst(j), xp, A.Identity,
                                                 bias=brow(j))
                        else:
                            nc.scalar.copy(dst(j), xp)
                    else:
                        e = nc.vector if eng == "v" else nc.gpsimd
                        if brow is not None:
                            e.tensor_scalar_add(dst(j), xp, brow(j))
                        else:
                            e.tensor_copy(dst(j), xp)

            # =======================================================
            # ENCODER
            # =======================================================
            with ExitStack() as ectx:
                epool = ectx.enter_context(tc.tile_pool(name="enc", bufs=1))
                psg_e = ectx.enter_context(
                    tc.tile_pool(name="psge", bufs=2, space="PSUM"))
                ew_ih = epool.tile([128, 128, 128], BF)
                ew_hh = epool.tile([128, 64, 128], BF)
                ew_b = epool.tile([128, 32], FP)
                nc.sync.dma_start(out=ew_ih, in_=d_ewih)
                nc.sync.dma_start(out=ew_hh, in_=d_ewhh)
                nc.sync.dma_start(out=ew_b, in_=d_eb)

                eo_bf = epool.tile([128, 4, S, NL], BF)
                for kk in range(4):
                    nc.sync.dma_start(out=eo_bf[:, kk], in_=d_encrhs[kk])

                # history tiles only for layer 0 (consumed by layer 1)
                e_hist = [epool.tile([128, S + 1, 2, NL], BF, name=f"eh{d}")
                          for d in range(2)]

                escan = {}
                for (l, d) in [(0, 0), (0, 1), (1, 0), (1, 1)]:
                    tag = f"e{l}{d}"
                    si = l * 2 + d
                    cst = epool.tile([128, 2, NL], FP, name=f"c_{tag}")
                    hp = [epool.tile([128, 2, NL], BF, name=f"hp{u}_{tag}")
                          for u in range(2)]
                    xsb = epool.tile([128, 8, C, NL], BF, name=f"x_{tag}")
                    nc.vector.memset(cst, 0.0)
                    nc.vector.memset(hp[0], 0.0)
                    escan[tag] = dict(
                        tag=tag, fwd=(d == 0), cst=cst, hp=hp, xsb=xsb,
                        whh_base=si * 16, wih_base=si * 32, b_base=si * 8,
                        hist=e_hist[d] if l == 0 else None)

                def enc_xtilde(sc, l, d, k):
                    tr0 = k * C if d == 0 else S - (k + 1) * C
                    srcs = []
                    for kk in range(4):
                        if l == 0:
                            rhs = eo_bf[:, kk, tr0:tr0 + C, :]
                        elif kk < 2:  # fwd outputs of L0: slot t+1
                            rhs = e_hist[0][:, tr0 + 1:tr0 + C + 1, kk, :]
                        else:         # bwd outputs of L0: slot t
                            rhs = e_hist[1][:, tr0:tr0 + C, kk - 2, :]
                        srcs.append((
                            (lambda kk_: lambda j, _:
                             ew_ih[:, sc["wih_base"] + kk_ * 8 + j, :])(kk),
                            rhs))
                    emit_xtilde(
                        srcs,
                        lambda j, b=sc["b_base"]: ew_b[:, b + j:b + j + 1],
                        lambda j: sc["xsb"][:, j, :, :],
                        copy_eng=["s", "v"])

                def enc_gates(sc, k, i, u):
                    """matmuls + sigma/tanh for one scan, one step."""
                    lt = i * 16 + u if sc["fwd"] else (C - 1) - (i * 16 + u)
                    gp = psg_e.tile([128, 8, NL], FP, tag=f"g_{sc['tag'][-1]}",
                                    bufs=3)
                    nc.tensor.matmul(gp, ident,
                                     sc["xsb"][:, :, ds(lt, 1), :],
                                     start=True, stop=False)
                    hr = sc["hp"][u % 2]
                    wb = sc["whh_base"]
                    for kk in range(2):
                        for j in range(8):
                            nc.tensor.matmul(
                                gp[:, j, :], ew_hh[:, wb + kk * 8 + j, :],
                                hr[:, kk, :], start=False,
                                stop=(kk == 1 and j == 7))
                    sifo = stash.tile([128, 6, NL], FP, tag=f"sf_{sc['tag']}")
                    tg = stash.tile([128, 2, NL], FP, tag=f"tg_{sc['tag']}")
                    nc.scalar.activation(sifo, gp[:, 0:6, :], A.Sigmoid)
                    nc.scalar.activation(tg, gp[:, 6:8, :], A.Tanh)
                    sc["_sifo"], sc["_tg"] = sifo, tg

                def enc_cupd(sc):
                    sifo, tg = sc["_sifo"], sc["_tg"]
                    t1 = stash.tile([128, 2, NL], FP, tag=f"t1_{sc['tag']}")
                    t2 = stash.tile([128, 2, NL], FP, tag=f"t2_{sc['tag']}")
                    cst = sc["cst"]
                    nc.vector.tensor_mul(t1, sifo[:, 0:2, :], tg)
                    nc.vector.tensor_mul(t2, sifo[:, 2:4, :], cst)
                    nc.vector.tensor_add(cst, t1, t2)

                def enc_tcl(sc):
                    tcl = stash.tile([128, 2, NL], FP, tag=f"tc_{sc['tag']}")
                    nc.scalar.activation(tcl, sc["cst"], A.Tanh)
                    sc["_tcl"] = tcl

                def enc_hout(sc, k, i, u):
                    hw = sc["hp"][1 - u % 2]
                    nc.vector.tensor_mul(hw, sc["_sifo"][:, 4:6, :], sc["_tcl"])
                    if sc["hist"] is not None:
                        t0 = k * C
                        if sc["fwd"]:
                            wslot = t0 + i * 16 + u + 1
                        else:
                            wslot = S - 1 - t0 - (i * 16 + u)
                        nc.gpsimd.tensor_copy(
                            sc["hist"][:, ds(wslot, 1), :, :], hw)

                for l in range(2):
                    scans = [escan[f"e{l}0"], escan[f"e{l}1"]]
                    for k in range(SC):
                        for d, sc in enumerate(scans):
                            enc_xtilde(sc, l, d, k)
                        with tc.For_i(0, C // 16, 1, staggered_reset=False) as i:
                            for u in range(16):
                                # fully sequential per scan: scan B's PE
                                # burst overlaps scan A's Act/DVE tail, so
                                # the two chains run half-step offset
                                for sc in scans:
                                    enc_gates(sc, k, i, u)
                                    enc_cupd(sc)
                                    enc_tcl(sc)
                                    enc_hout(sc, k, i, u)

                for li, tag in enumerate(["e00", "e01", "e10", "e11"]):
                    nc.vector.tensor_copy(hinit[li], escan[tag]["hp"][0])
                    nc.vector.tensor_copy(cinit[li], escan[tag]["cst"])

            # =======================================================
            # DECODER (4-layer chunk-lagged wavefront, tails merged in
            # groups of 2 layers)
            # =======================================================
            with ExitStack() as dctx:
                dpool = dctx.enter_context(tc.tile_pool(name="dec", bufs=1))
                psg_d = dctx.enter_context(
                    tc.tile_pool(name="psgd", bufs=2, space="PSUM"))
                dw_ih0 = dpool.tile([97, 8, 128], BF)
                dw_ih = dpool.tile([128, 48, 128], BF)
                dw_b = dpool.tile([128, 24], FP)
                dw_hh = dpool.tile([128, 64, 128], BF)
                fw = dpool.tile([128, 2, NMEL], BF)
                fb = dpool.tile([NMEL, 1], FP)
                nc.sync.dma_start(out=dw_ih0, in_=d_dwih0)
                nc.sync.dma_start(out=dw_ih, in_=d_dwih)
                nc.sync.dma_start(out=dw_b, in_=d_db)
                nc.sync.dma_start(out=dw_hh, in_=d_dwhh)
                nc.sync.dma_start(out=fw, in_=d_fcw)
                nc.sync.dma_start(out=fb, in_=d_fcb)

                teach = dpool.tile([97, CB], BF)
                nc.vector.memset(teach, 0.0)
                nc.vector.memset(teach[96:97, :], 1.0)  # bias row
                mst2 = [dpool.tile([NMEL, C, NL], FP, name=f"mst{z}")
                        for z in range(2)]
                icst = dpool.tile([NMEL, 1], FP)

                # shared state tiles across the 4 layer-scans
                hseq = dpool.tile([128, 4, TD + 1, 2, NL], BF, name="hseq_u")
                cst_d = dpool.tile([128, 4, 2, NL], FP, name="cst_d")
                hp_d = [dpool.tile([128, 4, 2, NL], BF, name=f"hpd{u}")
                        for u in range(2)]
                # (layer, j) fused so the shared ident matmul's rhs AP has
                # the same 3-free-dim shape as the per-scan case
                xsb_u = dpool.tile([128, 4 * 8, C, NL], BF, name="xsb_u")
                for l in range(4):
                    nc.vector.tensor_copy(cst_d[:, l], cinit[l])
                    nc.vector.tensor_copy(hp_d[0][:, l], hinit[l])

                def fill_teacher(k):
                    cur = mst2[k % 2]
                    if k == 0:
                        nc.sync.dma_start(
                            out=icst, in_=d_ictx.rearrange("(c o) -> c o", o=1))
                        nc.vector.tensor_copy(
                            cur[:, 0, :], icst.to_broadcast((NMEL, NL)))
                        nc.sync.dma_start(
                            out=cur[:, 1:, :], in_=d_mels[:, 0:C - 1, :])
                    if k + 1 < DC:  # prefetch next chunk during this phase
                        nc.sync.dma_start(
                            out=mst2[(k + 1) % 2],
                            in_=d_mels[:, (k + 1) * C - 1:(k + 2) * C - 1, :])
                    nc.vector.tensor_copy(teach[0:NMEL, :], cur)

                def dec_xtilde(l, k):
                    t0 = k * C
                    if l == 0:
                        emit_xtilde(
                            [(lambda j, _: dw_ih0[:, j, :], teach[:, :])],
                            None,
                            lambda j: xsb_u[:, j, :, :],
                            copy_eng=["s", "v"])
                    else:
                        srcs = []
                        for kk in range(2):
                            srcs.append((
                                (lambda kk_: lambda j, _:
                                 dw_ih[:, (l - 1) * 16 + kk_ * 8 + j, :])(kk),
                                hseq[:, l - 1, t0 + 1:t0 + C + 1, kk, :]))
                        emit_xtilde(
                            srcs,
                            lambda j, b=(l - 1) * 8: dw_b[:, b + j:b + j + 1],
                            lambda j: xsb_u[:, l * 8 + j, :, :],
                            copy_eng=["s", "v"])

                # one tail-group: layers glo..ghi (<=2), one step
                def dec_gates(glo, ghi, gi, i, u):
                    ng = ghi - glo + 1
                    lt = i * 16 + u
                    gp = psg_d.tile([128, 4, 8, NL], FP, tag=f"gd{gi}", bufs=3)
                    nc.tensor.matmul(
                        gp[:, 0:ng], ident,
                        xsb_u[:, glo * 8:(ghi + 1) * 8, ds(lt, 1), :],
                        start=True, stop=False)
                    for s in range(ng):
                        l = glo + s
                        hr = hp_d[u % 2]
                        for kk in range(2):
                            for j in range(8):
                                nc.tensor.matmul(
                                    gp[:, s, j, :],
                                    dw_hh[:, l * 16 + kk * 8 + j, :],
                                    hr[:, l, kk, :], start=False,
                                    stop=(s == ng - 1 and kk == 1 and j == 7))
                    sifo = stash.tile([128, 4, 6, NL], FP, tag=f"sfd{gi}")
                    tg = stash.tile([128, 4, 2, NL], FP, tag=f"tgd{gi}")
                    nc.scalar.activation(sifo[:, 0:ng], gp[:, 0:ng, 0:6, :],
                                         A.Sigmoid)
                    nc.scalar.activation(tg[:, 0:ng], gp[:, 0:ng, 6:8, :],
                                         A.Tanh)
                    return dict(glo=glo, ng=ng, gi=gi, sifo=sifo, tg=tg)

                def dec_cupd(g):
                    glo, ng, gi = g["glo"], g["ng"], g["gi"]
                    t1 = stash.tile([128, 4, 2, NL], FP, tag=f"t1d{gi}")
                    t2 = stash.tile([128, 4, 2, NL], FP, tag=f"t2d{gi}")
                    cs = cst_d[:, glo:glo + ng]
                    nc.vector.tensor_mul(t1[:, 0:ng], g["sifo"][:, 0:ng, 0:2, :],
                                         g["tg"][:, 0:ng])
                    nc.vector.tensor_mul(t2[:, 0:ng], g["sifo"][:, 0:ng, 2:4, :],
                                         cs)
                    nc.vector.tensor_add(cs, t1[:, 0:ng], t2[:, 0:ng])

                def dec_tcl(g):
                    glo, ng, gi = g["glo"], g["ng"], g["gi"]
                    tcl = stash.tile([128, 4, 2, NL], FP, tag=f"tcd{gi}")
                    nc.scalar.activation(tcl[:, 0:ng], cst_d[:, glo:glo + ng],
                                         A.Tanh)
                    g["tcl"] = tcl

                def dec_hout(g, p, i, u):
                    glo, ng = g["glo"], g["ng"]
                    hw = hp_d[1 - u % 2][:, glo:glo + ng]
                    nc.vector.tensor_mul(hw, g["sifo"][:, 0:ng, 4:6, :],
                                         g["tcl"][:, 0:ng])
                    for s in range(ng):
                        l = glo + s
                        wslot = (p - l) * C + i * 16 + u + 1
                        nc.gpsimd.tensor_copy(
                            hseq[:, l, ds(wslot, 1), :, :],
                            hp_d[1 - u % 2][:, l])

                def emit_fc(k):
                    t0 = k * C
                    fp = psum_x.tile([NMEL, CB], FP, tag="xp", name=f"fcp{k}")
                    for kk in range(2):
                        nc.tensor.matmul(
                            fp, fw[:, kk, :],
                            hseq[:, 3, t0 + 1:t0 + C + 1, kk, :],
                            start=(kk == 0), stop=(kk == 1))
                    fst = stash.tile([NMEL, C, NL], FP, tag="fst",
                                     name=f"fst{k}")
                    nc.scalar.activation(fst, fp.rearrange("p (t b) -> p t b", b=NL),
                                         A.Identity, bias=fb)
                    if k == DC - 1:
                        nc.vector.memset(fst[:, C - 1:C, :], 0.0)
                    nc.sync.dma_start(out=d_out[:, t0:t0 + C, :], in_=fst)

                for p in range(DC + 3):
                    active = [l for l in range(4) if 0 <= p - l < DC]
                    lo, hi = min(active), max(active)
                    for l in active:
                        if l == 0:
                            fill_teacher(p)
                        dec_xtilde(l, p - l)
                    GSZ = 2
                    groups = []
                    g0 = lo
                    while g0 <= hi:
                        g1 = min(g0 + GSZ - 1, hi)
                        groups.append((g0, g1, (g0 - lo) // GSZ))
                        g0 = g1 + 1
                    with tc.For_i(0, C // 16, 1, staggered_reset=False) as i:
                        for u in range(16):
                            # sequential per group -> half-step offset chains
                            for (a, b, gi) in groups:
                                g = dec_gates(a, b, gi, i, u)
                                dec_cupd(g)
                                dec_tcl(g)
                                dec_hout(g, p, i, u)
                    if 0 <= p - 4 < DC:
                        emit_fc(p - 4)

                fc_done = {p - 4 for p in range(DC + 3) if 0 <= p - 4 < DC}
                for k in range(DC):
                    if k not in fc_done:
                        emit_fc(k)

    # pin act tables to the one set with sigmoid+tanh+copy so the compiler
    # never emits in-loop table reloads
    import concourse.bacc as _bacc_mod
    _orig = _bacc_mod.get_activation_tables

    def _pinned(arch):
        t = _orig(arch)
        need = {A.Sigmoid, A.Tanh, A.Copy, A.Identity}
        # preserve set order/length: ids are indices into act_info.json.
        # make every set that does NOT fully cover our funcs unusable for
        # them, so the chooser lands on one set for all activations.
        out = {}
        for k, v in t.items():
            if need <= v:
                out[k] = v
            else:
                out[k] = v - need
        return out if any(need <= v for v in out.values()) else t

    _bacc_mod.get_activation_tables = _pinned
    try:
        nc.compile()
    finally:
        _bacc_mod.get_activation_tables = _orig
    return nc


def _host_prep(inputs):
    """Slice batch across cores + pre-tile/cast weights (gate-permuted)."""
    bf16 = ml_dtypes.bfloat16

    def tiles_T(w, kchunks, jchunks):
        # w: [4H, D] fp32 -> [k, j] of w.T tiles [128, 128] bf16, j gate-permuted
        wT = np.ascontiguousarray(w.T).astype(bf16)  # [D, 4H]
        out = np.zeros((kchunks, jchunks, 128, 128), bf16)
        for k in range(kchunks):
            for j in range(jchunks):
                pj = GPERM[j] if jchunks == 8 else j
                out[k, j] = wT[k * 128:(k + 1) * 128, pj * 128:(pj + 1) * 128]
        return out

    enc_Wih = np.asarray(inputs["enc_Wih"], np.float32)
    enc_Whh = np.asarray(inputs["enc_Whh"], np.float32)
    enc_b = np.asarray(inputs["enc_b"], np.float32)
    dec_Wih0 = np.asarray(inputs["dec_Wih0"], np.float32)
    dec_Wih = np.asarray(inputs["dec_Wih"], np.float32)
    dec_Whh = np.asarray(inputs["dec_Whh"], np.float32)
    dec_b = np.asarray(inputs["dec_b"], np.float32)
    fc_W = np.asarray(inputs["fc_W"], np.float32)
    fc_b = np.asarray(inputs["fc_b"], np.float32)
    ictx = np.asarray(inputs["init_ctx"], np.float32).reshape(-1)

    ewih = np.zeros((2, 2, 4, 8, 128, 128), bf16)
    ewhh = np.zeros((2, 2, 2, 8, 128, 128), bf16)
    eb = np.zeros((2, 2, 8, 128), np.float32)
    for l in range(2):
        for d in range(2):
            ewih[l, d] = tiles_T(enc_Wih[l, d], 4, 8)
            ewhh[l, d] = tiles_T(enc_Whh[l, d], 2, 8)
            eb[l, d] = enc_b[l, d].reshape(8, 128)[GPERM]

    dwih0 = np.zeros((97, 8, 128), bf16)
    w0T = dec_Wih0.T.astype(bf16)  # [80, 1024]
    b0 = dec_b[0].reshape(8, 128)
    for j in range(8):
        pj = GPERM[j]
        dwih0[0:80, j] = w0T[:, pj * 128:(pj + 1) * 128]
        dwih0[96, j] = b0[pj].astype(bf16)

    dwih = np.zeros((3, 2, 8, 128, 128), bf16)
    db = np.zeros((3, 8, 128), np.float32)
    for l in range(3):
        dwih[l] = tiles_T(dec_Wih[l], 2, 8)
        db[l] = dec_b[l + 1].reshape(8, 128)[GPERM]
    dwhh = np.zeros((4, 2, 8, 128, 128), bf16)
    for l in range(4):
        dwhh[l] = tiles_T(dec_Whh[l], 2, 8)

    fcw = np.zeros((2, 128, NMEL), bf16)
    fWT = fc_W.T.astype(bf16)  # [256, 80]
    fcw[0] = fWT[0:128]
    fcw[1] = fWT[128:256]

    encout = np.asarray(inputs["encoder_outputs"], np.float32)
    mels = np.asarray(inputs["mels"], np.float32)
    N = encout.shape[0]
    nb = N // NCORES

    base = {
        "ictx": ictx,
        "ewih": np.ascontiguousarray(ewih.reshape(128, 128, 128).transpose(1, 0, 2)),
        "ewhh": np.ascontiguousarray(ewhh.reshape(64, 128, 128).transpose(1, 0, 2)),
        "eb": np.ascontiguousarray(eb.reshape(32, 128).T),
        "dwih0": dwih0,
        "dwih": np.ascontiguousarray(dwih.reshape(48, 128, 128).transpose(1, 0, 2)),
        "db": np.ascontiguousarray(db.reshape(24, 128).T),
        "dwhh": np.ascontiguousarray(dwhh.reshape(64, 128, 128).transpose(1, 0, 2)),
        "fcw": np.ascontiguousarray(fcw.transpose(1, 0, 2)),
        "fcb": fc_b.astype(np.float32).reshape(NMEL, 1),
    }
    S = encout.shape[1]
    in_maps = []
    for cid in range(NCORES):
        m = dict(base)
        eo = encout[cid * nb:(cid + 1) * nb]  # [nb, S, 512]
        m["encrhs"] = np.ascontiguousarray(
            eo.transpose(2, 1, 0).reshape(4, 128, S, nb).astype(bf16))
        # mels: [nb, NMEL, T] -> [NMEL, T, nb]
        m["mels"] = np.ascontiguousarray(
            mels[cid * nb:(cid + 1) * nb].transpose(1, 2, 0))
        in_maps.append(m)
    return in_maps


def kernel(encoder_outputs, mels, text_lengths, output_lengths,
           enc_Wih, enc_Whh, enc_b, dec_Wih0, dec_Wih, dec_Whh, dec_b,
           fc_W, fc_b, init_ctx):
    from concourse import bass_utils

    inputs = dict(encoder_outputs=encoder_outputs, mels=mels,
                  enc_Wih=enc_Wih, enc_Whh=enc_Whh, enc_b=enc_b,
                  dec_Wih0=dec_Wih0, dec_Wih=dec_Wih, dec_Whh=dec_Whh,
                  dec_b=dec_b, fc_W=fc_W, fc_b=fc_b, init_ctx=init_ctx)
    N, S, _ = np.asarray(encoder_outputs).shape
    T = np.asarray(mels).shape[2]
    key = (S, T)
    if key not in _prog_cache:
        _prog_cache[key] = _build_program(S, T)
    nc = _prog_cache[key]
    in_maps = _host_prep(inputs)
    res = bass_utils.run_bass_kernel_spmd(nc, in_maps, core_ids=list(range(NCORES)))
    nb = N // NCORES
    out = np.zeros((N, NMEL, T), np.float32)
    for cid in range(NCORES):
        # device out: [NMEL, T, nb] -> [nb, NMEL, T]
        out[cid * nb:(cid + 1) * nb] = res.results[cid]["out"].transpose(2, 0, 1)
    return (out,)


# revision 23
# speedup vs baseline: 1.2169x; 1.0143x over previous
"""Trainium2 Bass kernel for nn_Decoder (Tacotron-style LSTM encoder/decoder).

Architecture (8 NeuronCores, data-parallel over batch N=64 -> 8/core):
  - Transposed layout: hidden dim on SBUF partitions, (time,batch) on the
    free dim; input-side projections batched into large matmuls per 64-step
    chunk; only h @ Whh.T stays per-step.
  - Recurrent h state lives in ping-pong tiles with STATIC addresses (the
    2-step-unrolled For_i body alternates them), so the 16 per-step h-matmuls
    need no dynamic address computation on the PE sequencer.  The h history
    (consumed at chunk granularity by the next layer / FC) is written by the
    otherwise-idle GpSimd engine, off the recurrent critical path.
  - Gate order is host-permuted to [i,f,o,g] so one sigmoid covers i,f,o and
    one tanh covers g: 3 activation instructions per step instead of 4.
  - Activation tables are pinned to the single set containing sigmoid+tanh+
    copy, eliminating per-iteration act-table reloads.
  - Decoder runs a 4-layer chunk-lagged wavefront; the elementwise tails are
    merged across groups of 2 layers (one instruction covers both layers'
    gates), halving tail instruction count while keeping 2 independent
    dependency chains in flight.
  - Weights / h / x-projections in bf16, cell state c + PSUM in fp32.
"""

import numpy as np
import ml_dtypes

H = 256
NMEL = 80
D_ENC = 512
NCORES = 8
NL = 8          # batch per core
C = 64          # chunk (time) size

# new j-block order [i0,i1,f0,f1,o0,o1,g0,g1] <- old [i,i,f,f,g,g,o,o]
GPERM = [0, 1, 2, 3, 6, 7, 4, 5]

_prog_cache = {}


def _build_program(S, T):
    import concourse.bass as bass
    import concourse.mybir as mybir
    import concourse.tile as tile
    import concourse.bacc as bacc_mod
    from concourse import bacc
    from concourse.bass import ds
    from concourse.masks import make_identity
    from contextlib import ExitStack

    BF = mybir.dt.bfloat16
    FP = mybir.dt.float32
    A = mybir.ActivationFunctionType

    TD = T
    assert S % C == 0 and TD % C == 0
    SC = S // C
    DC = TD // C

    nc = bacc.Bacc("TRN2", target_bir_lowering=False, debug=False,
                   num_devices=NCORES)

    # ---------------- DRAM I/O ----------------
    d_encrhs = nc.dram_tensor("encrhs", [4, 128, S, NL], BF, kind="ExternalInput").ap()
    d_mels = nc.dram_tensor("mels", [NMEL, T, NL], FP, kind="ExternalInput").ap()
    d_ictx = nc.dram_tensor("ictx", [NMEL], FP, kind="ExternalInput").ap()
    d_ewih = nc.dram_tensor("ewih", [128, 128, 128], BF, kind="ExternalInput").ap()
    d_ewhh = nc.dram_tensor("ewhh", [128, 64, 128], BF, kind="ExternalInput").ap()
    d_eb = nc.dram_tensor("eb", [128, 32], FP, kind="ExternalInput").ap()
    d_dwih0 = nc.dram_tensor("dwih0", [97, 8, 128], BF, kind="ExternalInput").ap()
    d_dwih = nc.dram_tensor("dwih", [128, 48, 128], BF, kind="ExternalInput").ap()
    d_db = nc.dram_tensor("db", [128, 24], FP, kind="ExternalInput").ap()
    d_dwhh = nc.dram_tensor("dwhh", [128, 64, 128], BF, kind="ExternalInput").ap()
    d_fcw = nc.dram_tensor("fcw", [128, 2, NMEL], BF, kind="ExternalInput").ap()
    d_fcb = nc.dram_tensor("fcb", [NMEL, 1], FP, kind="ExternalInput").ap()
    d_out = nc.dram_tensor("out", [NMEL, T, NL], FP, kind="ExternalOutput").ap()

    CB = C * NL  # tokens per chunk = 512

    with tile.TileContext(nc) as tc:
        with ExitStack() as ctx:
            persist = ctx.enter_context(tc.tile_pool(name="persist", bufs=1))
            psum_x = ctx.enter_context(
                tc.tile_pool(name="psx", bufs=2, space="PSUM"))
            stash = ctx.enter_context(tc.tile_pool(name="stash", bufs=3))

            ident = persist.tile([128, 128], BF)
            make_identity(nc, ident)
            hinit = [persist.tile([128, 2, NL], BF, name=f"hinit{l}")
                     for l in range(4)]
            cinit = [persist.tile([128, 2, NL], FP, name=f"cinit{l}")
                     for l in range(4)]

            # ---------- batched input projection for one chunk ----------
            def emit_xtilde(xsrcs, brow, dst, copy_eng):
                """xsrcs: list of (lhsT_ap, rhs_fn) accumulated; brow:
                fn j -> [128,1] bias column folded into the psum->sbuf copy
                (saves an N=512 bias matmul per j); dst(j) -> [128, C*NL]."""
                nk = len(xsrcs)
                for j in range(8):
                    xp = psum_x.tile([128, CB], FP, tag="xp")
                    for kk, (wap, rhs) in enumerate(xsrcs):
                        nc.tensor.matmul(xp, wap(j, kk), rhs,
                                         start=(kk == 0), stop=(kk == nk - 1))
                    eng = copy_eng[j % len(copy_eng)]
                    if eng == "s":
                        if brow is not None:
                            nc.scalar.activation(dst(j), xp, A.Identity,
                                                 bias=brow(j))
                        else:
                            nc.scalar.copy(dst(j), xp)
                    else:
                        e = nc.vector if eng == "v" else nc.gpsimd
                        if brow is not None:
                            e.tensor_scalar_add(dst(j), xp, brow(j))
                        else:
                            e.tensor_copy(dst(j), xp)

            # =======================================================
            # ENCODER
            # =======================================================
            with ExitStack() as ectx:
                epool = ectx.enter_context(tc.tile_pool(name="enc", bufs=1))
                psg_e = ectx.enter_context(
                    tc.tile_pool(name="psge", bufs=2, space="PSUM"))
                ew_ih = epool.tile([128, 128, 128], BF)
                ew_hh = epool.tile([128, 64, 128], BF)
                ew_b = epool.tile([128, 32], FP)
                nc.sync.dma_start(out=ew_ih, in_=d_ewih)
                nc.sync.dma_start(out=ew_hh, in_=d_ewhh)
                nc.sync.dma_start(out=ew_b, in_=d_eb)

                eo_bf = epool.tile([128, 4, S, NL], BF)
                for kk in range(4):
                    nc.sync.dma_start(out=eo_bf[:, kk], in_=d_encrhs[kk])

                # history tiles only for layer 0 (consumed by layer 1)
                e_hist = [epool.tile([128, S + 1, 2, NL], BF, name=f"eh{d}")
                          for d in range(2)]

                escan = {}
                for (l, d) in [(0, 0), (0, 1), (1, 0), (1, 1)]:
                    tag = f"e{l}{d}"
                    si = l * 2 + d
                    cst = epool.tile([128, 2, NL], FP, name=f"c_{tag}")
                    hp = [epool.tile([128, 2, NL], BF, name=f"hp{u}_{tag}")
                          for u in range(2)]
                    xsb = epool.tile([128, 8, C, NL], BF, name=f"x_{tag}")
                    nc.vector.memset(cst, 0.0)
                    nc.vector.memset(hp[0], 0.0)
                    escan[tag] = dict(
                        tag=tag, fwd=(d == 0), cst=cst, hp=hp, xsb=xsb,
                        whh_base=si * 16, wih_base=si * 32, b_base=si * 8,
                        hist=e_hist[d] if l == 0 else None)

                def enc_xtilde(sc, l, d, k):
                    tr0 = k * C if d == 0 else S - (k + 1) * C
                    srcs = []
                    for kk in range(4):
                        if l == 0:
                            rhs = eo_bf[:, kk, tr0:tr0 + C, :]
                        elif kk < 2:  # fwd outputs of L0: slot t+1
                            rhs = e_hist[0][:, tr0 + 1:tr0 + C + 1, kk, :]
                        else:         # bwd outputs of L0: slot t
                            rhs = e_hist[1][:, tr0:tr0 + C, kk - 2, :]
                        srcs.append((
                            (lambda kk_: lambda j, _:
                             ew_ih[:, sc["wih_base"] + kk_ * 8 + j, :])(kk),
                            rhs))
                    emit_xtilde(
                        srcs,
                        lambda j, b=sc["b_base"]: ew_b[:, b + j:b + j + 1],
                        lambda j: sc["xsb"][:, j, :, :],
                        copy_eng=["s", "v"])

                def enc_gates(sc, k, i, u):
                    """matmuls + sigma/tanh for one scan, one step."""
                    lt = i * 32 + u if sc["fwd"] else (C - 1) - (i * 32 + u)
                    gp = psg_e.tile([128, 8, NL], FP, tag=f"g_{sc['tag'][-1]}",
                                    bufs=3)
                    nc.tensor.matmul(gp, ident,
                                     sc["xsb"][:, :, ds(lt, 1), :],
                                     start=True, stop=False)
                    hr = sc["hp"][u % 2]
                    wb = sc["whh_base"]
                    for kk in range(2):
                        for j in range(8):
                            nc.tensor.matmul(
                                gp[:, j, :], ew_hh[:, wb + kk * 8 + j, :],
                                hr[:, kk, :], start=False,
                                stop=(kk == 1 and j == 7))
                    sifo = stash.tile([128, 6, NL], FP, tag=f"sf_{sc['tag']}")
                    tg = stash.tile([128, 2, NL], FP, tag=f"tg_{sc['tag']}")
                    nc.scalar.activation(sifo, gp[:, 0:6, :], A.Sigmoid)
                    nc.scalar.activation(tg, gp[:, 6:8, :], A.Tanh)
                    sc["_sifo"], sc["_tg"] = sifo, tg

                def enc_cupd(sc):
                    sifo, tg = sc["_sifo"], sc["_tg"]
                    t1 = stash.tile([128, 2, NL], FP, tag=f"t1_{sc['tag']}")
                    t2 = stash.tile([128, 2, NL], FP, tag=f"t2_{sc['tag']}")
                    cst = sc["cst"]
                    nc.vector.tensor_mul(t1, sifo[:, 0:2, :], tg)
                    nc.vector.tensor_mul(t2, sifo[:, 2:4, :], cst)
                    nc.vector.tensor_add(cst, t1, t2)

                def enc_tcl(sc):
                    tcl = stash.tile([128, 2, NL], FP, tag=f"tc_{sc['tag']}")
                    nc.scalar.activation(tcl, sc["cst"], A.Tanh)
                    sc["_tcl"] = tcl

                def enc_hout(sc, k, i, u):
                    hw = sc["hp"][1 - u % 2]
                    nc.vector.tensor_mul(hw, sc["_sifo"][:, 4:6, :], sc["_tcl"])
                    if sc["hist"] is not None:
                        t0 = k * C
                        if sc["fwd"]:
                            wslot = t0 + i * 32 + u + 1
                        else:
                            wslot = S - 1 - t0 - (i * 32 + u)
                        nc.gpsimd.tensor_copy(
                            sc["hist"][:, ds(wslot, 1), :, :], hw)

                for l in range(2):
                    scans = [escan[f"e{l}0"], escan[f"e{l}1"]]
                    for k in range(SC):
                        for d, sc in enumerate(scans):
                            enc_xtilde(sc, l, d, k)
                        with tc.For_i(0, C // 32, 1, staggered_reset=False) as i:
                            for u in range(32):
                                # fully sequential per scan: scan B's PE
                                # burst overlaps scan A's Act/DVE tail, so
                                # the two chains run half-step offset
                                for sc in scans:
                                    enc_gates(sc, k, i, u)
                                    enc_cupd(sc)
                                    enc_tcl(sc)
                                    enc_hout(sc, k, i, u)

                for li, tag in enumerate(["e00", "e01", "e10", "e11"]):
                    nc.vector.tensor_copy(hinit[li], escan[tag]["hp"][0])
                    nc.vector.tensor_copy(cinit[li], escan[tag]["cst"])

            # =======================================================
            # DECODER (4-layer chunk-lagged wavefront, tails merged in
            # groups of 2 layers)
            # =======================================================
            with ExitStack() as dctx:
                dpool = dctx.enter_context(tc.tile_pool(name="dec", bufs=1))
                psg_d = dctx.enter_context(
                    tc.tile_pool(name="psgd", bufs=2, space="PSUM"))
                dw_ih0 = dpool.tile([97, 8, 128], BF)
                dw_ih = dpool.tile([128, 48, 128], BF)
                dw_b = dpool.tile([128, 24], FP)
                dw_hh = dpool.tile([128, 64, 128], BF)
                fw = dpool.tile([128, 2, NMEL], BF)
                fb = dpool.tile([NMEL, 1], FP)
                nc.sync.dma_start(out=dw_ih0, in_=d_dwih0)
                nc.sync.dma_start(out=dw_ih, in_=d_dwih)
                nc.sync.dma_start(out=dw_b, in_=d_db)
                nc.sync.dma_start(out=dw_hh, in_=d_dwhh)
                nc.sync.dma_start(out=fw, in_=d_fcw)
                nc.sync.dma_start(out=fb, in_=d_fcb)

                teach = dpool.tile([97, CB], BF)
                nc.vector.memset(teach, 0.0)
                nc.vector.memset(teach[96:97, :], 1.0)  # bias row
                mst2 = [dpool.tile([NMEL, C, NL], FP, name=f"mst{z}")
                        for z in range(2)]
                icst = dpool.tile([NMEL, 1], FP)

                # shared state tiles across the 4 layer-scans
                hseq = dpool.tile([128, 4, TD + 1, 2, NL], BF, name="hseq_u")
                cst_d = dpool.tile([128, 4, 2, NL], FP, name="cst_d")
                hp_d = [dpool.tile([128, 4, 2, NL], BF, name=f"hpd{u}")
                        for u in range(2)]
                # (layer, j) fused so the shared ident matmul's rhs AP has
                # the same 3-free-dim shape as the per-scan case
                xsb_u = dpool.tile([128, 4 * 8, C, NL], BF, name="xsb_u")
                for l in range(4):
                    nc.vector.tensor_copy(cst_d[:, l], cinit[l])
                    nc.vector.tensor_copy(hp_d[0][:, l], hinit[l])

                def fill_teacher(k):
                    cur = mst2[k % 2]
                    if k == 0:
                        nc.sync.dma_start(
                            out=icst, in_=d_ictx.rearrange("(c o) -> c o", o=1))
                        nc.vector.tensor_copy(
                            cur[:, 0, :], icst.to_broadcast((NMEL, NL)))
                        nc.sync.dma_start(
                            out=cur[:, 1:, :], in_=d_mels[:, 0:C - 1, :])
                    if k + 1 < DC:  # prefetch next chunk during this phase
                        nc.sync.dma_start(
                            out=mst2[(k + 1) % 2],
                            in_=d_mels[:, (k + 1) * C - 1:(k + 2) * C - 1, :])
                    nc.vector.tensor_copy(teach[0:NMEL, :], cur)

                def dec_xtilde(l, k):
                    t0 = k * C
                    if l == 0:
                        emit_xtilde(
                            [(lambda j, _: dw_ih0[:, j, :], teach[:, :])],
                            None,
                            lambda j: xsb_u[:, j, :, :],
                            copy_eng=["s", "v"])
                    else:
                        srcs = []
                        for kk in range(2):
                            srcs.append((
                                (lambda kk_: lambda j, _:
                                 dw_ih[:, (l - 1) * 16 + kk_ * 8 + j, :])(kk),
                                hseq[:, l - 1, t0 + 1:t0 + C + 1, kk, :]))
                        emit_xtilde(
                            srcs,
                            lambda j, b=(l - 1) * 8: dw_b[:, b + j:b + j + 1],
                            lambda j: xsb_u[:, l * 8 + j, :, :],
                            copy_eng=["s", "v"])

                # one tail-group: layers glo..ghi (<=2), one step
                def dec_gates(glo, ghi, gi, i, u):
                    ng = ghi - glo + 1
                    lt = i * 32 + u
                    gp = psg_d.tile([128, 4, 8, NL], FP, tag=f"gd{gi}", bufs=3)
                    nc.tensor.matmul(
                        gp[:, 0:ng], ident,
                        xsb_u[:, glo * 8:(ghi + 1) * 8, ds(lt, 1), :],
                        start=True, stop=False)
                    for s in range(ng):
                        l = glo + s
                        hr = hp_d[u % 2]
                        for kk in range(2):
                            for j in range(8):
                                nc.tensor.matmul(
                                    gp[:, s, j, :],
                                    dw_hh[:, l * 16 + kk * 8 + j, :],
                                    hr[:, l, kk, :], start=False,
                                    stop=(s == ng - 1 and kk == 1 and j == 7))
                    sifo = stash.tile([128, 4, 6, NL], FP, tag=f"sfd{gi}")
                    tg = stash.tile([128, 4, 2, NL], FP, tag=f"tgd{gi}")
                    nc.scalar.activation(sifo[:, 0:ng], gp[:, 0:ng, 0:6, :],
                                         A.Sigmoid)
                    nc.scalar.activation(tg[:, 0:ng], gp[:, 0:ng, 6:8, :],
                                         A.Tanh)
                    return dict(glo=glo, ng=ng, gi=gi, sifo=sifo, tg=tg)

                def dec_cupd(g):
                    glo, ng, gi = g["glo"], g["ng"], g["gi"]
                    t1 = stash.tile([128, 4, 2, NL], FP, tag=f"t1d{gi}")
                    t2 = stash.tile([128, 4, 2, NL], FP, tag=f"t2d{gi}")
                    cs = cst_d[:, glo:glo + ng]
                    nc.vector.tensor_mul(t1[:, 0:ng], g["sifo"][:, 0:ng, 0:2, :],
                                         g["tg"][:, 0:ng])
                    nc.vector.tensor_mul(t2[:, 0:ng], g["sifo"][:, 0:ng, 2:4, :],
                                         cs)
                    nc.vector.tensor_add(cs, t1[:, 0:ng], t2[:, 0:ng])

                def dec_tcl(g):
                    glo, ng, gi = g["glo"], g["ng"], g["gi"]
                    tcl = stash.tile([128, 4, 2, NL], FP, tag=f"tcd{gi}")
                    nc.scalar.activation(tcl[:, 0:ng], cst_d[:, glo:glo + ng],
                                         A.Tanh)
                    g["tcl"] = tcl

                def dec_hout(g, p, i, u):
                    glo, ng = g["glo"], g["ng"]
                    hw = hp_d[1 - u % 2][:, glo:glo + ng]
                    nc.vector.tensor_mul(hw, g["sifo"][:, 0:ng, 4:6, :],
                                         g["tcl"][:, 0:ng])
                    for s in range(ng):
                        l = glo + s
                        wslot = (p - l) * C + i * 32 + u + 1
                        nc.gpsimd.tensor_copy(
                            hseq[:, l, ds(wslot, 1), :, :],
                            hp_d[1 - u % 2][:, l])

                def emit_fc(k):
                    t0 = k * C
                    fp = psum_x.tile([NMEL, CB], FP, tag="xp", name=f"fcp{k}")
                    for kk in range(2):
                        nc.tensor.matmul(
                            fp, fw[:, kk, :],
                            hseq[:, 3, t0 + 1:t0 + C + 1, kk, :],
                            start=(kk == 0), stop=(kk == 1))
                    fst = stash.tile([NMEL, C, NL], FP, tag="fst",
                                     name=f"fst{k}")
                    nc.scalar.activation(fst, fp.rearrange("p (t b) -> p t b", b=NL),
                                         A.Identity, bias=fb)
                    if k == DC - 1:
                        nc.vector.memset(fst[:, C - 1:C, :], 0.0)
                    nc.sync.dma_start(out=d_out[:, t0:t0 + C, :], in_=fst)

                for p in range(DC + 3):
                    active = [l for l in range(4) if 0 <= p - l < DC]
                    lo, hi = min(active), max(active)
                    for l in active:
                        if l == 0:
                            fill_teacher(p)
                        dec_xtilde(l, p - l)
                    GSZ = 2
                    groups = []
                    g0 = lo
                    while g0 <= hi:
                        g1 = min(g0 + GSZ - 1, hi)
                        groups.append((g0, g1, (g0 - lo) // GSZ))
                        g0 = g1 + 1
                    with tc.For_i(0, C // 32, 1, staggered_reset=False) as i:
                        for u in range(32):
                            # sequential per group -> half-step offset chains
                            for (a, b, gi) in groups:
                                g = dec_gates(a, b, gi, i, u)
                                dec_cupd(g)
                                dec_tcl(g)
                                dec_hout(g, p, i, u)
                    if 0 <= p - 4 < DC:
                        emit_fc(p - 4)

                fc_done = {p - 4 for p in range(DC + 3) if 0 <= p - 4 < DC}
                for k in range(DC):
                    if k not in fc_done:
                        emit_fc(k)

    # pin act tables to the one set with sigmoid+tanh+copy so the compiler
    # never emits in-loop table reloads
    import concourse.bacc as _bacc_mod
    _orig = _bacc_mod.get_activation_tables

    def _pinned(arch):
        t = _orig(arch)
        need = {A.Sigmoid, A.Tanh, A.Copy, A.Identity}
        # preserve set order/length: ids are indices into act_info.json.
        # make every set that does NOT fully cover our funcs unusable for
        # them, so the chooser lands on one set for all activations.
        out = {}
        for k, v in t.items():
            if need <= v:
                out[k] = v
            else:
                out[k] = v - need
        return out if any(need <= v for v in out.values()) else t

    _bacc_mod.get_activation_tables = _pinned
    try:
        nc.compile()
    finally:
        _bacc_mod.get_activation_tables = _orig
    return nc


def _host_prep(inputs):
    """Slice batch across cores + pre-tile/cast weights (gate-permuted)."""
    bf16 = ml_dtypes.bfloat16

    def tiles_T(w, kchunks, jchunks):
        # w: [4H, D] fp32 -> [k, j] of w.T tiles [128, 128] bf16, j gate-permuted
        wT = np.ascontiguousarray(w.T).astype(bf16)  # [D, 4H]
        out = np.zeros((kchunks, jchunks, 128, 128), bf16)
        for k in range(kchunks):
            for j in range(jchunks):
                pj = GPERM[j] if jchunks == 8 else j
                out[k, j] = wT[k * 128:(k + 1) * 128, pj * 128:(pj + 1) * 128]
        return out

    enc_Wih = np.asarray(inputs["enc_Wih"], np.float32)
    enc_Whh = np.asarray(inputs["enc_Whh"], np.float32)
    enc_b = np.asarray(inputs["enc_b"], np.float32)
    dec_Wih0 = np.asarray(inputs["dec_Wih0"], np.float32)
    dec_Wih = np.asarray(inputs["dec_Wih"], np.float32)
    dec_Whh = np.asarray(inputs["dec_Whh"], np.float32)
    dec_b = np.asarray(inputs["dec_b"], np.float32)
    fc_W = np.asarray(inputs["fc_W"], np.float32)
    fc_b = np.asarray(inputs["fc_b"], np.float32)
    ictx = np.asarray(inputs["init_ctx"], np.float32).reshape(-1)

    ewih = np.zeros((2, 2, 4, 8, 128, 128), bf16)
    ewhh = np.zeros((2, 2, 2, 8, 128, 128), bf16)
    eb = np.zeros((2, 2, 8, 128), np.float32)
    for l in range(2):
        for d in range(2):
            ewih[l, d] = tiles_T(enc_Wih[l, d], 4, 8)
            ewhh[l, d] = tiles_T(enc_Whh[l, d], 2, 8)
            eb[l, d] = enc_b[l, d].reshape(8, 128)[GPERM]

    dwih0 = np.zeros((97, 8, 128), bf16)
    w0T = dec_Wih0.T.astype(bf16)  # [80, 1024]
    b0 = dec_b[0].reshape(8, 128)
    for j in range(8):
        pj = GPERM[j]
        dwih0[0:80, j] = w0T[:, pj * 128:(pj + 1) * 128]
        dwih0[96, j] = b0[pj].astype(bf16)

    dwih = np.zeros((3, 2, 8, 128, 128), bf16)
    db = np.zeros((3, 8, 128), np.float32)
    for l in range(3):
        dwih[l] = tiles_T(dec_Wih[l], 2, 8)
        db[l] = dec_b[l + 1].reshape(8, 128)[GPERM]
    dwhh = np.zeros((4, 2, 8, 128, 128), bf16)
    for l in range(4):
        dwhh[l] = tiles_T(dec_Whh[l], 2, 8)

    fcw = np.zeros((2, 128, NMEL), bf16)
    fWT = fc_W.T.astype(bf16)  # [256, 80]
    fcw[0] = fWT[0:128]
    fcw[1] = fWT[128:256]

    encout = np.asarray(inputs["encoder_outputs"], np.float32)
    mels = np.asarray(inputs["mels"], np.float32)
    N = encout.shape[0]
    nb = N // NCORES

    base = {
        "ictx": ictx,
        "ewih": np.ascontiguousarray(ewih.reshape(128, 128, 128).transpose(1, 0, 2)),
        "ewhh": np.ascontiguousarray(ewhh.reshape(64, 128, 128).transpose(1, 0, 2)),
        "eb": np.ascontiguousarray(eb.reshape(32, 128).T),
        "dwih0": dwih0,
        "dwih": np.ascontiguousarray(dwih.reshape(48, 128, 128).transpose(1, 0, 2)),
        "db": np.ascontiguousarray(db.reshape(24, 128).T),
        "dwhh": np.ascontiguousarray(dwhh.reshape(64, 128, 128).transpose(1, 0, 2)),
        "fcw": np.ascontiguousarray(fcw.transpose(1, 0, 2)),
        "fcb": fc_b.astype(np.float32).reshape(NMEL, 1),
    }
    S = encout.shape[1]
    in_maps = []
    for cid in range(NCORES):
        m = dict(base)
        eo = encout[cid * nb:(cid + 1) * nb]  # [nb, S, 512]
        m["encrhs"] = np.ascontiguousarray(
            eo.transpose(2, 1, 0).reshape(4, 128, S, nb).astype(bf16))
        # mels: [nb, NMEL, T] -> [NMEL, T, nb]
        m["mels"] = np.ascontiguousarray(
            mels[cid * nb:(cid + 1) * nb].transpose(1, 2, 0))
        in_maps.append(m)
    return in_maps


def kernel(encoder_outputs, mels, text_lengths, output_lengths,
           enc_Wih, enc_Whh, enc_b, dec_Wih0, dec_Wih, dec_Whh, dec_b,
           fc_W, fc_b, init_ctx):
    from concourse import bass_utils

    inputs = dict(encoder_outputs=encoder_outputs, mels=mels,
                  enc_Wih=enc_Wih, enc_Whh=enc_Whh, enc_b=enc_b,
                  dec_Wih0=dec_Wih0, dec_Wih=dec_Wih, dec_Whh=dec_Whh,
                  dec_b=dec_b, fc_W=fc_W, fc_b=fc_b, init_ctx=init_ctx)
    N, S, _ = np.asarray(encoder_outputs).shape
    T = np.asarray(mels).shape[2]
    key = (S, T)
    if key not in _prog_cache:
        _prog_cache[key] = _build_program(S, T)
    nc = _prog_cache[key]
    in_maps = _host_prep(inputs)
    res = bass_utils.run_bass_kernel_spmd(nc, in_maps, core_ids=list(range(NCORES)))
    nb = N // NCORES
    out = np.zeros((N, NMEL, T), np.float32)
    for cid in range(NCORES):
        # device out: [NMEL, T, nb] -> [nb, NMEL, T]
        out[cid * nb:(cid + 1) * nb] = res.results[cid]["out"].transpose(2, 0, 1)
    return (out,)
